# revision 40
# baseline (speedup 1.0000x reference)
"""GatedCrossAttention Trainium2 kernel, v2.

Three SPMD launches with host reshard between (host work is layout only:
slicing, transposes, dtype casts, concatenation — all reference math runs
on device):

  L0 (row-parallel): core c LayerNorms its 256 rows of query_feats and
     kv_feats_wt (stats + apply on device).  Removes the 8x-duplicated
     full-tensor LN of the old design.  Host transposes the gathered
     normalized activations into [128, kc, T] layout for L1.
  L1 (head-parallel): core c owns head c of the three primary attentions
     (cross, self, wt).  Projections consume host-pre-transposed qnT/kvnT
     (no on-device input transposes).  Scores -> one 2-bank exp per key
     block -> value-stationary PV (accumulating ctx^T in PSUM, 8x fewer
     matmul instructions than ex-stationary) -> PE transpose back ->
     per-partition 1/den normalize on DVE.  Outputs normalized per-head
     context slices, bf16.
  L2 (token-parallel): core c owns 256 token rows.  Gate MHA with a
     num/den matmul trick: PV emits gate-ctx^T per head; a tiny [65,2]
     matmul against (mvec_h | e64) yields per-token numerator and
     denominator of the mix logit directly, so the gate context is never
     normalized, copied, or transposed.  Then sigmoid mix, out-projection,
     and the gated FeedForward (batched 2-per-bank gelu).

All LayerNorm affine weights fold into downstream matmul weights
host-side, attention 1/sqrt(d) into q-side weights, ff_gate into fc2,
and mha_out_w + mix_w collapse into one vector (mvec).  Matmuls in bf16
with fp32 PSUM accumulation; softmax skips max subtraction (logits are
small, overflow impossible).
"""
import os
import sys
sys.path.insert(0, '/opt/trn_rl_repo')

import numpy as np
import ml_dtypes

import concourse.bass as bass
import concourse.bacc as bacc
import concourse.tile as tile
import concourse.mybir as mybir
from concourse.bass_utils import run_bass_kernel_spmd
from concourse.masks import make_identity

F32 = mybir.dt.float32
BF16 = mybir.dt.bfloat16
F8 = mybir.dt.float8e4
AF = mybir.ActivationFunctionType
ALU = mybir.AluOpType
AX = mybir.AxisListType

B, N, D = 2, 1024, 1024
H, DH = 8, 64
INNER = 512
FF = 4096
T = B * N            # 2048 flattened tokens
EPS = 1e-5
NCORES = 8
RPC = T // NCORES    # 256 rows per core (L0 / L2)


# ---------------------------------------------------------------- helpers
def _ln_rn(nc, norm, xt, ncols, eps_ap):
    """Stats for LayerNorm of xt [128, ncols]: returns (r, nb) f32 [128,1]
    with y = x*r + nb."""
    nsub = ncols // 512
    st = norm.tile([128, nsub, 6], F32, tag="st")
    for s in range(nsub):
        nc.vector.bn_stats(out=st[:, s, :], in_=xt[:, s * 512:(s + 1) * 512])
    mv = norm.tile([128, 2], F32, tag="mv")
    nc.vector.bn_aggr(out=mv, in_=st)
    sd = norm.tile([128, 1], F32, tag="sd")
    nc.scalar.activation(out=sd, in_=mv[:, 1:2], func=AF.Sqrt, bias=eps_ap)
    r = norm.tile([128, 1], F32, tag="r")
    nc.vector.reciprocal(out=r, in_=sd)
    nb = norm.tile([128, 1], F32, tag="nb")
    nc.vector.tensor_scalar(out=nb, in0=mv[:, 0:1], scalar1=r, scalar2=-1.0,
                            op0=ALU.mult, op1=ALU.mult)
    return r, nb


# ---------------------------------------------------------------- launch 0
def build_l0():
    nc = bacc.Bacc("TRN2", target_bir_lowering=False, debug=False,
                   num_devices=NCORES)
    xr = nc.dram_tensor("xr", [2 * RPC, D], BF16, kind="ExternalInput").ap()
    xo = nc.dram_tensor("xo", [2 * RPC, D], BF16, kind="ExternalOutput").ap()
    with tile.TileContext(nc) as tc:
        with tc.tile_pool(name="io", bufs=1) as io, \
             tc.tile_pool(name="norm", bufs=1) as norm:
            eps_ap = norm.tile([128, 1], F32, tag="eps")
            nc.vector.memset(eps_ap, EPS)
            xt = io.tile([128, 4, D], BF16, tag="xt")
            nc.gpsimd.dma_start(
                out=xt, in_=xr.rearrange("(j p) d -> p j d", p=128))
            ot = io.tile([128, 4, D], BF16, tag="ot")
            # batched stats for all 4 row tiles: one sqrt/recip round trip
            st = norm.tile([128, 4, 2, 6], F32, tag="st")
            for j in range(4):
                for s in range(2):
                    nc.vector.bn_stats(out=st[:, j, s, :],
                                       in_=xt[:, j, s * 512:(s + 1) * 512])
            mv4 = norm.tile([128, 4, 2], F32, tag="mv4")
            for j in range(4):
                nc.vector.bn_aggr(out=mv4[:, j, :], in_=st[:, j, :, :])
            sd4 = norm.tile([128, 4], F32, tag="sd4")
            nc.scalar.activation(out=sd4, in_=mv4[:, :, 1], func=AF.Sqrt,
                                 bias=eps_ap)
            r4 = norm.tile([128, 4], F32, tag="r4")
            nc.vector.reciprocal(out=r4, in_=sd4)
            nb4 = norm.tile([128, 4], F32, tag="nb4")
            nc.vector.tensor_tensor(out=nb4, in0=mv4[:, :, 0], in1=r4,
                                    op=ALU.mult)
            nc.vector.tensor_scalar_mul(out=nb4, in0=nb4, scalar1=-1.0)
            for j in range(4):
                if j % 2 == 0:
                    nc.vector.tensor_scalar(
                        out=ot[:, j, :], in0=xt[:, j, :],
                        scalar1=r4[:, j:j + 1], scalar2=nb4[:, j:j + 1],
                        op0=ALU.mult, op1=ALU.add)
                else:
                    nc.scalar.activation(out=ot[:, j, :], in_=xt[:, j, :],
                                         func=AF.Identity,
                                         bias=nb4[:, j:j + 1],
                                         scale=r4[:, j:j + 1])
            nc.sync.dma_start(
                out=xo.rearrange("(j p) d -> p j d", p=128), in_=ot)
    nc.compile()
    return nc


# ---------------------------------------------------------------- launch 1
def build_l1():
    nc = bacc.Bacc("TRN2", target_bir_lowering=False, debug=False,
                   num_devices=NCORES)
    qnT = nc.dram_tensor("qnT", [128, 8, T], BF16, kind="ExternalInput").ap()
    kvnT = nc.dram_tensor("kvnT", [128, 8, T], BF16, kind="ExternalInput").ap()
    p1w = nc.dram_tensor("p1w", [128, 8, 128], BF16, kind="ExternalInput").ap()
    p2w = nc.dram_tensor("p2w", [128, 8, 128], BF16, kind="ExternalInput").ap()
    p3w = nc.dram_tensor("p3w", [128, 8, 128], BF16, kind="ExternalInput").ap()
    p4w = nc.dram_tensor("p4w", [128, 8, 128], BF16, kind="ExternalInput").ap()
    p5w = nc.dram_tensor("p5w", [128, 8, 64], BF16, kind="ExternalInput").ap()
    # outputs in [p, tblock, d] layout; host reassembles rows
    self_o = nc.dram_tensor("self_o", [128, 16, DH], BF16,
                            kind="ExternalOutput").ap()
    cross_o = nc.dram_tensor("cross_o", [128, 16, DH], BF16,
                             kind="ExternalOutput").ap()
    wt_o = nc.dram_tensor("wt_o", [128, 16, DH], BF16,
                          kind="ExternalOutput").ap()

    KC = D // 128    # 8 channel chunks

    with tile.TileContext(nc) as tc:
        with tc.tile_pool(name="const", bufs=1) as const, \
             tc.tile_pool(name="wein", bufs=1) as wein, \
             tc.tile_pool(name="xin", bufs=1) as xin, \
             tc.tile_pool(name="projT", bufs=1) as projT:
            ident = const.tile([128, 128], BF16)
            make_identity(nc, ident)

            # weight DMAs first: the DMA queue drains in order, and the
            # first projection only needs w + the first input chunk
            wsbs = []
            for wdram, mwid in ((p1w, 128), (p2w, 128), (p3w, 128),
                                (p4w, 128), (p5w, 64)):
                wsb = wein.tile([128, KC, mwid], BF16,
                                tag=f"w{len(wsbs)}", name=f"w{len(wsbs)}")
                nc.sync.dma_start(out=wsb, in_=wdram)
                wsbs.append(wsb)

            qn_sb = xin.tile([128, KC, T], BF16)
            kvn_sb = xin.tile([128, KC, T], BF16)
            # the two big input loads go through different engine DMA queues
            # (sync issues ~1.2us per dma_start, serially) so they transfer
            # concurrently and don't queue behind each other
            nc.gpsimd.dma_start(out=qn_sb, in_=qnT)
            nc.scalar.dma_start(out=kvn_sb, in_=kvnT)

            p1T = projT.tile([128, T], BF16)   # [q_c | k_s]
            p2T = projT.tile([128, T], BF16)   # [v_s | q_s]
            p3T = projT.tile([128, T], BF16)   # [k_c | q_wt]
            p4T = projT.tile([128, T], BF16)   # [v_c | k_wt]
            p5T = projT.tile([64, T], BF16)    # v_wt

            # ---- projections (weight stationary held over 4 psum banks)
            with tc.tile_pool(name="psproj", bufs=2, space="PSUM") as psp:
                for wsb, xsb, dst, mwid in (
                        (wsbs[0], qn_sb, p1T, 128),
                        (wsbs[1], qn_sb, p2T, 128),
                        (wsbs[2], kvn_sb, p3T, 128),
                        (wsbs[3], kvn_sb, p4T, 128),
                        (wsbs[4], kvn_sb, p5T, 64)):
                    pps = [psp.tile([128, 512], F32, tag=f"pp{nb_}",
                                    name=f"pp{nb_}") for nb_ in range(4)]
                    for kc in range(KC):
                        for nb_ in range(4):
                            nc.tensor.matmul(
                                pps[nb_][:mwid, :], lhsT=wsb[:, kc, :],
                                rhs=xsb[:, kc, nb_ * 512:(nb_ + 1) * 512],
                                start=(kc == 0), stop=(kc == KC - 1))
                    for nb_ in range(4):
                        nc.vector.tensor_copy(
                            out=dst[:, nb_ * 512:(nb_ + 1) * 512],
                            in_=pps[nb_][:mwid, :])

            # ---- v transposes + ones column
            with tc.tile_pool(name="vaugp", bufs=1) as vaugp:
                vaug_c = vaugp.tile([128, 16, 65], BF16)
                vaug_s = vaugp.tile([128, 16, 65], BF16)
                vaug_w = vaugp.tile([128, 16, 65], BF16)
                with tc.tile_pool(name="pstr2", bufs=4,
                                  space="PSUM") as pstr2:
                    for srcT, vaug in ((p4T[0:64, :], vaug_c),
                                       (p2T[0:64, :], vaug_s),
                                       (p5T[0:64, :], vaug_w)):
                        nc.gpsimd.memset(vaug[:, :, 64:65], 1.0)
                        for kb in range(16):
                            pt2 = pstr2.tile([128, 64], BF16, tag="pt2")
                            nc.tensor.transpose(
                                pt2, srcT[:, kb * 128:(kb + 1) * 128],
                                ident[0:64, 0:64])
                            nc.vector.tensor_copy(out=vaug[:, kb, 0:64],
                                                  in_=pt2)

                # ---- attentions
                specs = (
                    (p1T[0:64, :], p3T[0:64, :], vaug_c, cross_o),
                    (p2T[64:128, :], p1T[64:128, :], vaug_s, self_o),
                    (p3T[64:128, :], p4T[64:128, :], vaug_w, wt_o),
                )
                with tc.tile_pool(name="expp", bufs=2) as expp, \
                     tc.tile_pool(name="ctsp", bufs=2) as ctsp, \
                     tc.tile_pool(name="smallp", bufs=8) as smallp, \
                     tc.tile_pool(name="ctxp", bufs=2) as ctxp, \
                     tc.tile_pool(name="pss", bufs=2, space="PSUM") as pss, \
                     tc.tile_pool(name="psc", bufs=2, space="PSUM") as psc, \
                     tc.tile_pool(name="pst", bufs=2, space="PSUM") as pst:
                    for qT, kT, vaug, odram in specs:
                        ctx_sb = ctxp.tile([128, 16, DH], BF16, tag="ctx")
                        for b in range(B):
                            ex = expp.tile([128, 8, N], BF16, tag="ex")
                            for kb in range(8):
                                gkb = b * 8 + kb
                                ss = pss.tile([128, N], F32, tag="ss")
                                for nq2 in range(2):
                                    nc.tensor.matmul(
                                        ss[:, nq2 * 512:(nq2 + 1) * 512],
                                        lhsT=kT[:, gkb * 128:(gkb + 1) * 128],
                                        rhs=qT[:, b * N + nq2 * 512:
                                               b * N + (nq2 + 1) * 512],
                                        start=True, stop=True)
                                nc.scalar.activation(
                                    out=ex[:, kb, :], in_=ss, func=AF.Exp)
                            for nq2 in range(2):
                                pc = psc.tile([65, 512], F32, tag="pc")
                                for kb in range(8):
                                    nc.tensor.matmul(
                                        pc, lhsT=vaug[:, b * 8 + kb, :],
                                        rhs=ex[:, kb,
                                               nq2 * 512:(nq2 + 1) * 512],
                                        start=(kb == 0), stop=(kb == 7))
                                cts = ctsp.tile([65, 512], BF16, tag="cts")
                                nc.vector.tensor_copy(out=cts, in_=pc)
                                for j in range(4):
                                    ptx = pst.tile([128, 65], BF16, tag="ptx")
                                    nc.tensor.transpose(
                                        ptx, cts[:, j * 128:(j + 1) * 128],
                                        ident[0:65, 0:65])
                                    rec = smallp.tile([128, 1], F32, tag="rec")
                                    nc.vector.reciprocal(out=rec,
                                                         in_=ptx[:, 64:65])
                                    qb = b * 8 + nq2 * 4 + j
                                    nc.vector.tensor_scalar(
                                        out=ctx_sb[:, qb, :],
                                        in0=ptx[:, 0:64], scalar1=rec,
                                        scalar2=None, op0=ALU.mult)
                        nc.sync.dma_start(out=odram, in_=ctx_sb)
    nc.compile()
    return nc


# ---------------------------------------------------------------- launch 2
def build_l2(bdiff: float):
    nc = bacc.Bacc("TRN2", target_bir_lowering=False, debug=False,
                   num_devices=NCORES)
    selfr = nc.dram_tensor("selfr", [RPC, INNER], BF16, kind="ExternalInput").ap()
    crossr = nc.dram_tensor("crossr", [RPC, INNER], BF16, kind="ExternalInput").ap()
    wtr = nc.dram_tensor("wtr", [RPC, INNER], BF16, kind="ExternalInput").ap()
    crossb = nc.dram_tensor("crossb", [N, INNER], BF16, kind="ExternalInput").ap()
    wqg8 = nc.dram_tensor("wqg8", [128, 4, INNER], F8, kind="ExternalInput").ap()
    wkg8 = nc.dram_tensor("wkg8", [128, 4, INNER], F8, kind="ExternalInput").ap()
    wvg8 = nc.dram_tensor("wvg8", [128, 4, INNER], F8, kind="ExternalInput").ap()
    mv2d = nc.dram_tensor("mv2d", [65, 8, 2], BF16, kind="ExternalInput").ap()
    woT = nc.dram_tensor("woT", [128, 4, D], BF16, kind="ExternalInput").ap()
    wf1T = nc.dram_tensor("wf1T", [8, 128, 8, 512], BF16, kind="ExternalInput").ap()
    wf2T = nc.dram_tensor("wf2T", [8, 128, 4, D], BF16, kind="ExternalInput").ap()
    outd = nc.dram_tensor("outd", [RPC, D], F32, kind="ExternalOutput").ap()
    outw = nc.dram_tensor("outw", [RPC, D], F32, kind="ExternalOutput").ap()

    GS = 1.0 / 64.0     # gate weights are shipped fp8 * 64
    KI = INNER // 128   # 4 chunks over INNER
    with tile.TileContext(nc) as tc:
        with tc.tile_pool(name="const", bufs=1) as const, \
             tc.tile_pool(name="wpre", bufs=1) as wpre, \
             tc.tile_pool(name="persist", bufs=1) as persist, \
             tc.tile_pool(name="norm", bufs=4) as norm:
            ident = const.tile([128, 128], BF16)
            make_identity(nc, ident)
            eps_ap = const.tile([128, 1], F32)
            nc.vector.memset(eps_ap, EPS)

            conT8 = persist.tile([128, KI, N], F8)
            sonT8 = persist.tile([128, KI, RPC], F8)
            wtrT = persist.tile([128, KI, RPC], BF16)
            selff = persist.tile([128, 2, INNER], BF16)   # raw self rows
            crossf = persist.tile([128, 2, INNER], BF16)  # raw cross rows

            # activation loads first — the DMA queue drains in order and
            # phase A needs these immediately
            xt4s = []
            for g in range(2):
                xt4 = wpre.tile([128, 4, INNER], BF16, tag=f"xt4{g}",
                                name=f"xt4{g}")
                nc.sync.dma_start(
                    out=xt4,
                    in_=crossb[g * 512:(g + 1) * 512, :].rearrange(
                        "(j p) d -> p j d", p=128))
                xt4s.append(xt4)
            nc.sync.dma_start(
                out=selff, in_=selfr.rearrange("(j p) d -> p j d", p=128))
            nc.sync.dma_start(
                out=crossf, in_=crossr.rearrange("(j p) d -> p j d", p=128))
            wtf = wpre.tile([128, 2, INNER], BF16)
            nc.sync.dma_start(
                out=wtf, in_=wtr.rearrange("(j p) d -> p j d", p=128))

            # weights go through the scalar/gpsimd queues so they stream
            # concurrently with the sync-queue activation loads above
            wo_sb = wpre.tile([128, KI, D], BF16)
            nc.scalar.dma_start(out=wo_sb, in_=woT)
            wk_sb = wpre.tile([128, KI, INNER], F8)
            nc.scalar.dma_start(out=wk_sb, in_=wkg8)
            wq_sb = wpre.tile([128, KI, INNER], F8)
            nc.scalar.dma_start(out=wq_sb, in_=wqg8)
            wv_sb = wpre.tile([128, KI, INNER], F8)
            nc.scalar.dma_start(out=wv_sb, in_=wvg8)
            mv_sb = wpre.tile([65, 8, 2], BF16)
            nc.scalar.dma_start(out=mv_sb, in_=mv2d)
            w1all = wpre.tile([128, 8, 8, 512], BF16)
            nc.gpsimd.dma_start(out=w1all,
                                in_=wf1T.rearrange("g p k n -> p g k n"))

            # ---- phase A: LN + transposes
            with tc.tile_pool(name="io", bufs=2) as io, \
                 tc.tile_pool(name="pstr", bufs=4, space="PSUM") as pstr:
                for g in range(2):
                    xt4 = xt4s[g]
                    for j in range(4):
                        tb = g * 4 + j
                        xb = io.tile([128, INNER], BF16, tag="xb")
                        r, nb = _ln_rn(nc, norm, xt4[:, j, :], INNER, eps_ap)
                        if j % 2 == 0:
                            nc.vector.tensor_scalar(
                                out=xb, in0=xt4[:, j, :], scalar1=r,
                                scalar2=nb, op0=ALU.mult, op1=ALU.add)
                        else:
                            nc.scalar.activation(out=xb, in_=xt4[:, j, :],
                                                 func=AF.Identity, bias=nb,
                                                 scale=r)
                        for kc in range(KI):
                            pt = pstr.tile([128, 128], BF16, tag="pt")
                            nc.tensor.transpose(
                                pt, xb[:, kc * 128:(kc + 1) * 128], ident)
                            nc.vector.tensor_copy(
                                out=conT8[:, kc, tb * 128:(tb + 1) * 128],
                                in_=pt)
                for qsb in range(2):
                    sb_ = io.tile([128, INNER], BF16, tag="xb")
                    r, nb = _ln_rn(nc, norm, selff[:, qsb, :], INNER, eps_ap)
                    nc.scalar.activation(out=sb_, in_=selff[:, qsb, :],
                                         func=AF.Identity, bias=nb, scale=r)
                    for kc in range(KI):
                        pt = pstr.tile([128, 128], BF16, tag="pt")
                        nc.tensor.transpose(
                            pt, sb_[:, kc * 128:(kc + 1) * 128], ident)
                        nc.scalar.copy(
                            out=sonT8[:, kc, qsb * 128:(qsb + 1) * 128], in_=pt)
                    for kc in range(KI):
                        pt = pstr.tile([128, 128], BF16, tag="pt")
                        nc.tensor.transpose(
                            pt, wtf[:, qsb, kc * 128:(kc + 1) * 128], ident)
                        nc.scalar.copy(
                            out=wtrT[:, kc, qsb * 128:(qsb + 1) * 128], in_=pt)

            # ---- wt out-projection (independent of the gate chain; emitted
            # early so the PE can fill gaps during gate attention)
            with tc.tile_pool(name="psw", bufs=2, space="PSUM") as psw, \
                 tc.tile_pool(name="outw_p", bufs=4) as outw_p:
                for qsb in range(2):
                    ppw = [psw.tile([128, 512], F32, tag=f"wo{nb_}",
                                    name=f"wo{nb_}") for nb_ in range(2)]
                    for kc in range(KI):
                        for nb_ in range(2):
                            nc.tensor.matmul(
                                ppw[nb_],
                                lhsT=wtrT[:, kc, qsb * 128:(qsb + 1) * 128],
                                rhs=wo_sb[:, kc, nb_ * 512:(nb_ + 1) * 512],
                                start=(kc == 0), stop=(kc == KI - 1))
                    for nb_ in range(2):
                        ow = outw_p.tile([128, 512], F32, tag="ow")
                        nc.scalar.copy(out=ow, in_=ppw[nb_])
                        nc.sync.dma_start(
                            out=outw[qsb * 128:(qsb + 1) * 128,
                                     nb_ * 512:(nb_ + 1) * 512],
                            in_=ow)

            # ---- phase B: gate projections (fp8, DoubleRow over kc pairs)
            DR = mybir.MatmulPerfMode.DoubleRow
            with tc.tile_pool(name="gproj", bufs=1) as gproj:
                kgT = gproj.tile([128, KI, N], BF16)
                qgT = gproj.tile([128, KI, RPC], BF16)
                vaug = gproj.tile([128, 8, H, 65], BF16)
                with tc.tile_pool(name="psb", bufs=2, space="PSUM") as psb:
                    for mo in range(KI):
                        pps = [psb.tile([128, 512], F32, tag=f"gp{nb_}",
                                        name=f"gp{nb_}") for nb_ in range(2)]
                        for kcp in range(2):
                            for nb_ in range(2):
                                nc.tensor.matmul(
                                    pps[nb_],
                                    lhsT=wk_sb[:, 2 * kcp:2 * kcp + 2,
                                               mo * 128:(mo + 1) * 128],
                                    rhs=conT8[:, 2 * kcp:2 * kcp + 2,
                                              nb_ * 512:(nb_ + 1) * 512],
                                    perf_mode=DR,
                                    start=(kcp == 0), stop=(kcp == 1))
                        for nb_ in range(2):
                            nc.vector.tensor_scalar(
                                out=kgT[:, mo, nb_ * 512:(nb_ + 1) * 512],
                                in0=pps[nb_], scalar1=GS, scalar2=None,
                                op0=ALU.mult)
                    for mo in range(KI):
                        ppq = psb.tile([128, RPC], F32, tag="gq")
                        for kcp in range(2):
                            nc.tensor.matmul(
                                ppq,
                                lhsT=wq_sb[:, 2 * kcp:2 * kcp + 2,
                                           mo * 128:(mo + 1) * 128],
                                rhs=sonT8[:, 2 * kcp:2 * kcp + 2, :],
                                perf_mode=DR,
                                start=(kcp == 0), stop=(kcp == 1))
                        nc.vector.tensor_scalar(
                            out=qgT[:, mo, :], in0=ppq, scalar1=GS,
                            scalar2=None, op0=ALU.mult)
                    nc.gpsimd.memset(vaug[:, :, :, 64:65], 1.0)
                    for kb in range(8):
                        pp = psb.tile([128, H, 64], F32, tag="gv")
                        for kcp in range(2):
                            nc.tensor.matmul(
                                pp,
                                lhsT=conT8[:, 2 * kcp:2 * kcp + 2,
                                           kb * 128:(kb + 1) * 128],
                                rhs=wv_sb[:, 2 * kcp:2 * kcp + 2, :],
                                perf_mode=DR,
                                start=(kcp == 0), stop=(kcp == 1))
                        nc.vector.tensor_scalar(
                            out=vaug[:, kb, :, 0:64], in0=pp, scalar1=GS,
                            scalar2=None, op0=ALU.mult)

                # ---- phase C: gate attention, ctx^T + num/den per head
                gctxT = gproj.tile([65, H, RPC], BF16)
                with tc.tile_pool(name="expg", bufs=2) as expg, \
                     tc.tile_pool(name="psg", bufs=2, space="PSUM") as psg, \
                     tc.tile_pool(name="psc", bufs=2, space="PSUM") as psc2:
                    for h in range(H):
                        mo, po = h // 2, (h % 2) * 64
                        ex = expg.tile([128, 8, RPC], BF16, tag="ex")
                        for kq in range(2):
                            ss = psg.tile([128, 4, RPC], F32, tag="ss")
                            for k4 in range(4):
                                kb = kq * 4 + k4
                                nc.tensor.matmul(
                                    ss[:, k4, :],
                                    lhsT=kgT[po:po + 64, mo,
                                             kb * 128:(kb + 1) * 128],
                                    rhs=qgT[po:po + 64, mo, :],
                                    start=True, stop=True)
                            nc.scalar.activation(
                                out=ex[:, kq * 4:(kq + 1) * 4, :], in_=ss,
                                func=AF.Exp)
                        pg = psc2.tile([65, RPC], F32, tag="pg")
                        for kb in range(8):
                            nc.tensor.matmul(
                                pg, lhsT=vaug[:, kb, h, :], rhs=ex[:, kb, :],
                                start=(kb == 0), stop=(kb == 7))
                        nc.vector.tensor_copy(out=gctxT[:, h, :], in_=pg)

                # ---- phase D: mix scalars + mixed + transposes
                mixedT = gproj.tile([128, KI, RPC], BF16)
                with tc.tile_pool(name="mixp", bufs=4) as mixp, \
                     tc.tile_pool(name="pstr3", bufs=4, space="PSUM") as pstr3, \
                     tc.tile_pool(name="psn", bufs=2, space="PSUM") as psn:
                    for qsb in range(2):
                        nd = psn.tile([128, H, 2], F32, tag="nd")
                        for h in range(H):
                            nc.tensor.matmul(
                                nd[:, h, :],
                                lhsT=gctxT[:, h, qsb * 128:(qsb + 1) * 128],
                                rhs=mv_sb[:, h, :], start=True, stop=True)
                        rden = mixp.tile([128, H], F32, tag="rden")
                        nc.vector.reciprocal(out=rden, in_=nd[:, :, 1])
                        tnum = mixp.tile([128, H], F32, tag="tnum")
                        nc.vector.tensor_tensor(out=tnum, in0=nd[:, :, 0],
                                                in1=rden, op=ALU.mult)
                        logit = mixp.tile([128, 1], F32, tag="logit")
                        nc.vector.tensor_reduce(out=logit, in_=tnum,
                                                axis=AX.X, op=ALU.add)
                        mix1 = mixp.tile([128, 1], F32, tag="mix1")
                        nc.scalar.activation(out=mix1, in_=logit,
                                             func=AF.Sigmoid,
                                             bias=float(bdiff), scale=1.0)
                        mix0 = mixp.tile([128, 1], F32, tag="mix0")
                        nc.scalar.activation(out=mix0, in_=logit,
                                             func=AF.Sigmoid,
                                             bias=float(-bdiff), scale=-1.0)
                        t2 = mixp.tile([128, INNER], F32, tag="t2")
                        nc.vector.tensor_scalar_mul(
                            out=t2, in0=crossf[:, qsb, :], scalar1=mix1)
                        mixed_bf = mixp.tile([128, INNER], BF16, tag="mixed")
                        nc.vector.scalar_tensor_tensor(
                            out=mixed_bf, in0=selff[:, qsb, :], scalar=mix0,
                            in1=t2, op0=ALU.mult, op1=ALU.add)
                        for kc in range(KI):
                            pt = pstr3.tile([128, 128], BF16, tag="pt")
                            nc.tensor.transpose(
                                pt, mixed_bf[:, kc * 128:(kc + 1) * 128],
                                ident)
                            nc.scalar.copy(
                                out=mixedT[:, kc, qsb * 128:(qsb + 1) * 128],
                                in_=pt)

                # ---- phase E: delta out-projection
                delta = gproj.tile([128, 2, D], F32)
                with tc.tile_pool(name="pse", bufs=2, space="PSUM") as pse:
                    for qsb in range(2):
                        pps = [pse.tile([128, 512], F32, tag=f"eo{nb_}",
                                        name=f"eo{nb_}")
                               for nb_ in range(2)]
                        for kc in range(KI):
                            for nb_ in range(2):
                                nc.tensor.matmul(
                                    pps[nb_],
                                    lhsT=mixedT[:, kc,
                                                qsb * 128:(qsb + 1) * 128],
                                    rhs=wo_sb[:, kc,
                                              nb_ * 512:(nb_ + 1) * 512],
                                    start=(kc == 0), stop=(kc == KI - 1))
                        for nb_ in range(2):
                            nc.vector.tensor_copy(
                                out=delta[:, qsb,
                                          nb_ * 512:(nb_ + 1) * 512],
                                in_=pps[nb_])

                # ---- phase F: FeedForward
                with tc.tile_pool(name="ffp", bufs=1) as ffp, \
                     tc.tile_pool(name="io2", bufs=3) as io2, \
                     tc.tile_pool(name="psf", bufs=2, space="PSUM") as psf:
                    yT = ffp.tile([128, 8, RPC], BF16)
                    for qsb in range(2):
                        yb = io2.tile([128, D], BF16, tag="yb")
                        r, nb = _ln_rn(nc, norm, delta[:, qsb, :], D, eps_ap)
                        nc.scalar.activation(out=yb, in_=delta[:, qsb, :],
                                             func=AF.Identity, bias=nb,
                                             scale=r)
                        for kc in range(8):
                            pt = psf.tile([128, 128], BF16, tag="pt")
                            nc.tensor.transpose(
                                pt, yb[:, kc * 128:(kc + 1) * 128], ident)
                            nc.scalar.copy(
                                out=yT[:, kc, qsb * 128:(qsb + 1) * 128],
                                in_=pt)
                    h1T = ffp.tile([128, 32, RPC], BF16)
                    with tc.tile_pool(name="psh", bufs=2, space="PSUM") as psh:
                        for mog in range(8):
                            for mo2 in range(2):
                                ph = psh.tile([128, 2, RPC], F32, tag="ph")
                                for mi in range(2):
                                    mo = mo2 * 2 + mi
                                    for kc in range(8):
                                        nc.tensor.matmul(
                                            ph[:, mi, :],
                                            lhsT=w1all[:, mog, kc,
                                                       mo * 128:(mo + 1) * 128],
                                            rhs=yT[:, kc, :],
                                            start=(kc == 0), stop=(kc == 7))
                                nc.scalar.activation(
                                    out=h1T[:, mog * 4 + mo2 * 2:
                                            mog * 4 + mo2 * 2 + 2, :],
                                    in_=ph, func=AF.Gelu)
                    with tc.tile_pool(name="wf2p", bufs=4) as wf2p, \
                         tc.tile_pool(name="psy", bufs=1, space="PSUM") as psy, \
                         tc.tile_pool(name="outd_p", bufs=4) as outd_p:
                        pys = [[psy.tile([128, 512], F32, tag=f"py{q}{n}",
                                         name=f"py{q}{n}")
                                for n in range(2)] for q in range(2)]
                        for g2 in range(8):
                            w2 = wf2p.tile([128, 4, D], BF16, tag="w2")
                            nc.sync.dma_start(out=w2, in_=wf2T[g2])
                            for mo in range(4):
                                mo32 = g2 * 4 + mo
                                for qsb in range(2):
                                    for nb_ in range(2):
                                        nc.tensor.matmul(
                                            pys[qsb][nb_],
                                            lhsT=h1T[:, mo32,
                                                     qsb * 128:(qsb + 1) * 128],
                                            rhs=w2[:, mo,
                                                   nb_ * 512:(nb_ + 1) * 512],
                                            start=(mo32 == 0), stop=(mo32 == 31))
                        for qsb in range(2):
                            for nb_ in range(2):
                                od = outd_p.tile([128, 512], F32, tag="od")
                                nc.vector.tensor_tensor(
                                    out=od, in0=pys[qsb][nb_],
                                    in1=delta[:, qsb, nb_ * 512:(nb_ + 1) * 512],
                                    op=ALU.add)
                                nc.sync.dma_start(
                                    out=outd[qsb * 128:(qsb + 1) * 128,
                                             nb_ * 512:(nb_ + 1) * 512],
                                    in_=od)
    nc.compile()
    return nc


# ---------------------------------------------------------------- host glue
_BUILT = {}
LAST_PROFILE = {}


def _get(key, builder, *args):
    if key not in _BUILT:
        _BUILT[key] = builder(*args)
    return _BUILT[key]


def _bf16(x):
    return np.ascontiguousarray(np.asarray(x).astype(ml_dtypes.bfloat16))


def _fp8(x):
    return np.ascontiguousarray(
        np.clip(np.asarray(x, np.float32), -448, 448).astype(
            ml_dtypes.float8_e4m3fn))


def _shuf(wT, kc):
    """[kc*128, m] -> [128, kc, m] so each SBUF partition row is contiguous."""
    m = wT.shape[1]
    return np.ascontiguousarray(wT.reshape(kc, 128, m).transpose(1, 0, 2))


def _run(nc, in_maps, tag):
    _trace = os.environ.get("KTRACE", "0") == "1"
    res = run_bass_kernel_spmd(nc, in_maps, core_ids=list(range(NCORES)),
                               trace=_trace)
    LAST_PROFILE[f"{tag}_ns"] = res.exec_time_ns
    if res.instructions_and_trace is not None:
        LAST_PROFILE[f"{tag}_trace"] = res.instructions_and_trace[1]
    return res


def kernel(query_feats, kv_feats_wt, nq_w, nq_b, nkv_w, nkv_b, wq_cross,
           wkv_cross, wqkv_self, gn_w, gn_b, mha_in_w, mha_out_w, mix_w,
           mix_b, w_out, ff_ln_w, ff_ln_b, ff_fc1, ff_fc2, ff_gate):
    f = lambda x: np.asarray(x, dtype=np.float32)
    query_feats, kv_feats_wt = f(query_feats), f(kv_feats_wt)
    nq_w, nq_b, nkv_w, nkv_b = f(nq_w), f(nq_b), f(nkv_w), f(nkv_b)
    wq_cross, wkv_cross, wqkv_self = f(wq_cross), f(wkv_cross), f(wqkv_self)
    gn_w, gn_b = f(gn_w), f(gn_b)
    mha_in_w, mha_out_w, mix_w, mix_b = f(mha_in_w), f(mha_out_w), f(mix_w), f(mix_b)
    w_out, ff_ln_w, ff_ln_b = f(w_out), f(ff_ln_w), f(ff_ln_b)
    ff_fc1, ff_fc2, ff_gate = f(ff_fc1), f(ff_fc2), f(ff_gate)

    for b_, nm in ((nq_b, "nq_b"), (nkv_b, "nkv_b"), (gn_b, "gn_b"),
                   (ff_ln_b, "ff_ln_b")):
        assert np.all(b_ == 0.0), f"{nm} != 0 unsupported by this kernel"

    scale = DH ** -0.5
    qf2 = _bf16(query_feats.reshape(T, D))
    kvf2 = _bf16(kv_feats_wt.reshape(T, D))

    # ---------------- launch 0: row-parallel LayerNorm
    nc0 = _get("l0", build_l0)
    in_maps0 = [{"xr": np.concatenate(
        [qf2[c * RPC:(c + 1) * RPC], kvf2[c * RPC:(c + 1) * RPC]], axis=0)}
        for c in range(NCORES)]
    res0 = _run(nc0, in_maps0, "l0")
    qn = np.concatenate(
        [res0.results[c]["xo"][:RPC] for c in range(NCORES)], axis=0)
    kvn = np.concatenate(
        [res0.results[c]["xo"][RPC:] for c in range(NCORES)], axis=0)
    # transpose to [128, kc, T] (pure layout, host-side)
    qnT = np.ascontiguousarray(
        qn.T.reshape(8, 128, T).transpose(1, 0, 2))
    kvnT = np.ascontiguousarray(
        kvn.T.reshape(8, 128, T).transpose(1, 0, 2))

    wq_self = wqkv_self[0:INNER]
    wk_self = wqkv_self[INNER:2 * INNER]
    wv_self = wqkv_self[2 * INNER:3 * INNER]
    wk_cross = wkv_cross[0:INNER]
    wv_cross = wkv_cross[INNER:2 * INNER]

    # ---------------- launch 1
    nc1 = _get("l1", build_l1)
    in_maps1 = []
    for c in range(NCORES):
        s = slice(c * DH, (c + 1) * DH)
        p1 = np.concatenate([
            (wq_cross[s] * nq_w[None, :] * scale).T,
            (wk_self[s] * nq_w[None, :]).T], axis=1)
        p2 = np.concatenate([
            (wv_self[s] * nq_w[None, :]).T,
            (wq_self[s] * nq_w[None, :] * scale).T], axis=1)
        p3 = np.concatenate([
            (wk_cross[s] * nkv_w[None, :]).T,
            (wq_self[s] * nkv_w[None, :] * scale).T], axis=1)
        p4 = np.concatenate([
            (wv_cross[s] * nkv_w[None, :]).T,
            (wk_self[s] * nkv_w[None, :]).T], axis=1)
        p5 = (wv_self[s] * nkv_w[None, :]).T
        in_maps1.append({
            "qnT": qnT, "kvnT": kvnT,
            "p1w": _bf16(_shuf(p1, 8)), "p2w": _bf16(_shuf(p2, 8)),
            "p3w": _bf16(_shuf(p3, 8)), "p4w": _bf16(_shuf(p4, 8)),
            "p5w": _bf16(_shuf(p5, 8)),
        })
    res1 = _run(nc1, in_maps1, "l1")
    # outputs come back [128, 16, 64]; rows t = tb*128 + p
    unblk = lambda a: np.ascontiguousarray(
        a.transpose(1, 0, 2).reshape(T, DH))
    self_out = np.concatenate(
        [unblk(res1.results[c]["self_o"]) for c in range(NCORES)], axis=1)
    cross_out = np.concatenate(
        [unblk(res1.results[c]["cross_o"]) for c in range(NCORES)], axis=1)
    wt_ctx = np.concatenate(
        [unblk(res1.results[c]["wt_o"]) for c in range(NCORES)], axis=1)

    # ---------------- launch 2
    wq_g = mha_in_w[0:INNER]
    wk_g = mha_in_w[INNER:2 * INNER]
    wv_g = mha_in_w[2 * INNER:3 * INNER]
    dmix = mix_w[1] - mix_w[0]
    bdiff = float(mix_b[1] - mix_b[0])
    mvec = (mha_out_w.T @ dmix).reshape(INNER)
    mv2 = np.zeros((65, 8, 2), np.float32)
    for h in range(H):
        mv2[0:64, h, 0] = mvec[h * 64:(h + 1) * 64]
        mv2[64, h, 1] = 1.0
    wqg8 = _fp8(_shuf((wq_g * gn_w[None, :] * scale).T * 64.0, 4))
    wkg8 = _fp8(_shuf((wk_g * gn_w[None, :]).T * 64.0, 4))
    wvg8 = _fp8(_shuf((wv_g * gn_w[None, :]).T * 64.0, 4))
    woT = _bf16(_shuf(w_out.T, 4))
    wf1s = (ff_fc1 * ff_ln_w[None, :]).T          # [D, FF]
    wf1s = wf1s.reshape(8, 128, 8, 512).transpose(2, 1, 0, 3)  # [g,p,kc,n]
    wf2s = (ff_fc2 * float(ff_gate.reshape(-1)[0])).T          # [FF, D]
    wf2s = wf2s.reshape(8, 4, 128, D).transpose(0, 2, 1, 3)    # [g,p,mo,n]

    nc2 = _get(("l2", bdiff), build_l2, bdiff)
    in_maps2 = []
    wf1sb = _bf16(wf1s)
    wf2sb = _bf16(wf2s)
    mv2b = _bf16(mv2)
    for c in range(NCORES):
        g0 = c * RPC
        bb = g0 // N
        in_maps2.append({
            "selfr": self_out[g0:g0 + RPC], "crossr": cross_out[g0:g0 + RPC],
            "wtr": wt_ctx[g0:g0 + RPC],
            "crossb": cross_out[bb * N:(bb + 1) * N],
            "wqg8": wqg8, "wkg8": wkg8, "wvg8": wvg8,
            "mv2d": mv2b, "woT": woT,
            "wf1T": wf1sb, "wf2T": wf2sb,
        })
    res2 = _run(nc2, in_maps2, "l2")
    delta = np.concatenate(
        [res2.results[c]["outd"] for c in range(NCORES)], axis=0)
    wt_out = np.concatenate(
        [res2.results[c]["outw"] for c in range(NCORES)], axis=0)

    return np.stack([delta.reshape(B, N, D),
                     wt_out.reshape(B, N, D)]).astype(np.float32)


# revision 43
# speedup vs baseline: 1.1264x; 1.1264x over previous
"""GatedCrossAttention Trainium2 kernel, v2.

Three SPMD launches with host reshard between (host work is layout only:
slicing, transposes, dtype casts, concatenation — all reference math runs
on device):

  L0 (row-parallel): core c LayerNorms its 256 rows of query_feats and
     kv_feats_wt (stats + apply on device).  Removes the 8x-duplicated
     full-tensor LN of the old design.  Host transposes the gathered
     normalized activations into [128, kc, T] layout for L1.
  L1 (head-parallel): core c owns head c of the three primary attentions
     (cross, self, wt).  Projections consume host-pre-transposed qnT/kvnT
     (no on-device input transposes).  Scores -> one 2-bank exp per key
     block -> value-stationary PV (accumulating ctx^T in PSUM, 8x fewer
     matmul instructions than ex-stationary) -> PE transpose back ->
     per-partition 1/den normalize on DVE.  Outputs normalized per-head
     context slices, bf16.
  L2 (token-parallel): core c owns 256 token rows.  Gate MHA with a
     num/den matmul trick: PV emits gate-ctx^T per head; a tiny [65,2]
     matmul against (mvec_h | e64) yields per-token numerator and
     denominator of the mix logit directly, so the gate context is never
     normalized, copied, or transposed.  Then sigmoid mix, out-projection,
     and the gated FeedForward (batched 2-per-bank gelu).

All LayerNorm affine weights fold into downstream matmul weights
host-side, attention 1/sqrt(d) into q-side weights, ff_gate into fc2,
and mha_out_w + mix_w collapse into one vector (mvec).  Matmuls in bf16
with fp32 PSUM accumulation; softmax skips max subtraction (logits are
small, overflow impossible).
"""
import os
import sys
sys.path.insert(0, '/opt/trn_rl_repo')

import numpy as np
import ml_dtypes

import concourse.bass as bass
import concourse.bacc as bacc
import concourse.tile as tile
import concourse.mybir as mybir
from concourse.bass_utils import run_bass_kernel_spmd
from concourse.masks import make_identity

F32 = mybir.dt.float32
BF16 = mybir.dt.bfloat16
F8 = mybir.dt.float8e4
AF = mybir.ActivationFunctionType
ALU = mybir.AluOpType
AX = mybir.AxisListType

B, N, D = 2, 1024, 1024
H, DH = 8, 64
INNER = 512
FF = 4096
T = B * N            # 2048 flattened tokens
EPS = 1e-5
NCORES = 8
RPC = T // NCORES    # 256 rows per core (L0 / L2)


# ---------------------------------------------------------------- helpers
def _ln_rn(nc, norm, xt, ncols, eps_ap):
    """Stats for LayerNorm of xt [128, ncols]: returns (r, nb) f32 [128,1]
    with y = x*r + nb."""
    nsub = ncols // 512
    st = norm.tile([128, nsub, 6], F32, tag="st")
    for s in range(nsub):
        nc.vector.bn_stats(out=st[:, s, :], in_=xt[:, s * 512:(s + 1) * 512])
    mv = norm.tile([128, 2], F32, tag="mv")
    nc.vector.bn_aggr(out=mv, in_=st)
    sd = norm.tile([128, 1], F32, tag="sd")
    nc.scalar.activation(out=sd, in_=mv[:, 1:2], func=AF.Sqrt, bias=eps_ap)
    r = norm.tile([128, 1], F32, tag="r")
    nc.vector.reciprocal(out=r, in_=sd)
    nb = norm.tile([128, 1], F32, tag="nb")
    nc.vector.tensor_scalar(out=nb, in0=mv[:, 0:1], scalar1=r, scalar2=-1.0,
                            op0=ALU.mult, op1=ALU.mult)
    return r, nb


# ---------------------------------------------------------------- launch 0
def build_l0():
    nc = bacc.Bacc("TRN2", target_bir_lowering=False, debug=False,
                   num_devices=NCORES)
    xr = nc.dram_tensor("xr", [2 * RPC, D], BF16, kind="ExternalInput").ap()
    xo = nc.dram_tensor("xo", [2 * RPC, D], BF16, kind="ExternalOutput").ap()
    with tile.TileContext(nc) as tc:
        with tc.tile_pool(name="io", bufs=1) as io, \
             tc.tile_pool(name="norm", bufs=1) as norm:
            eps_ap = norm.tile([128, 1], F32, tag="eps")
            nc.vector.memset(eps_ap, EPS)
            xt = io.tile([128, 4, D], BF16, tag="xt")
            nc.sync.dma_start(
                out=xt, in_=xr.rearrange("(j p) d -> p j d", p=128))
            ot = io.tile([128, 4, D], BF16, tag="ot")
            # batched stats for all 4 row tiles: one sqrt/recip round trip
            st = norm.tile([128, 4, 2, 6], F32, tag="st")
            for j in range(4):
                for s in range(2):
                    nc.vector.bn_stats(out=st[:, j, s, :],
                                       in_=xt[:, j, s * 512:(s + 1) * 512])
            mv4 = norm.tile([128, 4, 2], F32, tag="mv4")
            for j in range(4):
                nc.vector.bn_aggr(out=mv4[:, j, :], in_=st[:, j, :, :])
            sd4 = norm.tile([128, 4], F32, tag="sd4")
            nc.scalar.activation(out=sd4, in_=mv4[:, :, 1], func=AF.Sqrt,
                                 bias=eps_ap)
            r4 = norm.tile([128, 4], F32, tag="r4")
            nc.vector.reciprocal(out=r4, in_=sd4)
            nb4 = norm.tile([128, 4], F32, tag="nb4")
            nc.vector.tensor_tensor(out=nb4, in0=mv4[:, :, 0], in1=r4,
                                    op=ALU.mult)
            nc.vector.tensor_scalar_mul(out=nb4, in0=nb4, scalar1=-1.0)
            for j in range(4):
                if j % 2 == 0:
                    nc.vector.tensor_scalar(
                        out=ot[:, j, :], in0=xt[:, j, :],
                        scalar1=r4[:, j:j + 1], scalar2=nb4[:, j:j + 1],
                        op0=ALU.mult, op1=ALU.add)
                else:
                    nc.scalar.activation(out=ot[:, j, :], in_=xt[:, j, :],
                                         func=AF.Identity,
                                         bias=nb4[:, j:j + 1],
                                         scale=r4[:, j:j + 1])
            nc.sync.dma_start(
                out=xo.rearrange("(j p) d -> p j d", p=128), in_=ot)
    nc.compile()
    return nc


# ---------------------------------------------------------------- launch 1
def build_l1():
    nc = bacc.Bacc("TRN2", target_bir_lowering=False, debug=False,
                   num_devices=NCORES)
    qnT = nc.dram_tensor("qnT", [128, 8, T], BF16, kind="ExternalInput").ap()
    kvnT = nc.dram_tensor("kvnT", [128, 8, T], BF16, kind="ExternalInput").ap()
    p1w = nc.dram_tensor("p1w", [128, 8, 128], BF16, kind="ExternalInput").ap()
    p2w = nc.dram_tensor("p2w", [128, 8, 128], BF16, kind="ExternalInput").ap()
    p3w = nc.dram_tensor("p3w", [128, 8, 128], BF16, kind="ExternalInput").ap()
    p4w = nc.dram_tensor("p4w", [128, 8, 128], BF16, kind="ExternalInput").ap()
    p5w = nc.dram_tensor("p5w", [128, 8, 64], BF16, kind="ExternalInput").ap()
    # outputs in [p, tblock, d] layout; host reassembles rows
    self_o = nc.dram_tensor("self_o", [128, 16, DH], BF16,
                            kind="ExternalOutput").ap()
    cross_o = nc.dram_tensor("cross_o", [128, 16, DH], BF16,
                             kind="ExternalOutput").ap()
    wt_o = nc.dram_tensor("wt_o", [128, 16, DH], BF16,
                          kind="ExternalOutput").ap()

    KC = D // 128    # 8 channel chunks

    with tile.TileContext(nc) as tc:
        with tc.tile_pool(name="const", bufs=1) as const, \
             tc.tile_pool(name="wein", bufs=1) as wein, \
             tc.tile_pool(name="xin", bufs=1) as xin, \
             tc.tile_pool(name="projT", bufs=1) as projT:
            ident = const.tile([128, 128], BF16)
            make_identity(nc, ident)

            # weight DMAs first: the DMA queue drains in order, and the
            # first projection only needs w + the first input chunk
            wsbs = []
            for wdram, mwid in ((p1w, 128), (p2w, 128), (p3w, 128),
                                (p4w, 128), (p5w, 64)):
                wsb = wein.tile([128, KC, mwid], BF16,
                                tag=f"w{len(wsbs)}", name=f"w{len(wsbs)}")
                nc.sync.dma_start(out=wsb, in_=wdram)
                wsbs.append(wsb)

            qn_sb = xin.tile([128, KC, T], BF16)
            kvn_sb = xin.tile([128, KC, T], BF16)
            # column-chunked loads so projections start before the full
            # activations arrive
            for t4 in range(4):
                cs = slice(t4 * 512, (t4 + 1) * 512)
                nc.sync.dma_start(out=qn_sb[:, :, cs], in_=qnT[:, :, cs])
                nc.sync.dma_start(out=kvn_sb[:, :, cs], in_=kvnT[:, :, cs])

            p1T = projT.tile([128, T], BF16)   # [q_c | k_s]
            p2T = projT.tile([128, T], BF16)   # [v_s | q_s]
            p3T = projT.tile([128, T], BF16)   # [k_c | q_wt]
            p4T = projT.tile([128, T], BF16)   # [v_c | k_wt]
            p5T = projT.tile([64, T], BF16)    # v_wt

            # ---- projections (weight stationary held over 4 psum banks)
            with tc.tile_pool(name="psproj", bufs=2, space="PSUM") as psp:
                for wsb, xsb, dst, mwid in (
                        (wsbs[0], qn_sb, p1T, 128),
                        (wsbs[1], qn_sb, p2T, 128),
                        (wsbs[2], kvn_sb, p3T, 128),
                        (wsbs[3], kvn_sb, p4T, 128),
                        (wsbs[4], kvn_sb, p5T, 64)):
                    pps = [psp.tile([128, 512], F32, tag=f"pp{nb_}",
                                    name=f"pp{nb_}") for nb_ in range(4)]
                    for kc in range(KC):
                        for nb_ in range(4):
                            nc.tensor.matmul(
                                pps[nb_][:mwid, :], lhsT=wsb[:, kc, :],
                                rhs=xsb[:, kc, nb_ * 512:(nb_ + 1) * 512],
                                start=(kc == 0), stop=(kc == KC - 1))
                    for nb_ in range(4):
                        nc.vector.tensor_copy(
                            out=dst[:, nb_ * 512:(nb_ + 1) * 512],
                            in_=pps[nb_][:mwid, :])

            # ---- v transposes + ones column
            with tc.tile_pool(name="vaugp", bufs=1) as vaugp:
                vaug_c = vaugp.tile([128, 16, 65], BF16)
                vaug_s = vaugp.tile([128, 16, 65], BF16)
                vaug_w = vaugp.tile([128, 16, 65], BF16)
                with tc.tile_pool(name="pstr2", bufs=4,
                                  space="PSUM") as pstr2:
                    for srcT, vaug in ((p4T[0:64, :], vaug_c),
                                       (p2T[0:64, :], vaug_s),
                                       (p5T[0:64, :], vaug_w)):
                        nc.gpsimd.memset(vaug[:, :, 64:65], 1.0)
                        for kb in range(16):
                            pt2 = pstr2.tile([128, 64], BF16, tag="pt2")
                            nc.tensor.transpose(
                                pt2, srcT[:, kb * 128:(kb + 1) * 128],
                                ident[0:64, 0:64])
                            nc.vector.tensor_copy(out=vaug[:, kb, 0:64],
                                                  in_=pt2)

                # ---- attentions
                specs = (
                    (p1T[0:64, :], p3T[0:64, :], vaug_c, cross_o),
                    (p2T[64:128, :], p1T[64:128, :], vaug_s, self_o),
                    (p3T[64:128, :], p4T[64:128, :], vaug_w, wt_o),
                )
                with tc.tile_pool(name="expp", bufs=2) as expp, \
                     tc.tile_pool(name="ctsp", bufs=2) as ctsp, \
                     tc.tile_pool(name="smallp", bufs=8) as smallp, \
                     tc.tile_pool(name="ctxp", bufs=2) as ctxp, \
                     tc.tile_pool(name="pss", bufs=2, space="PSUM") as pss, \
                     tc.tile_pool(name="psc", bufs=2, space="PSUM") as psc, \
                     tc.tile_pool(name="pst", bufs=2, space="PSUM") as pst:
                    for qT, kT, vaug, odram in specs:
                        ctx_sb = ctxp.tile([128, 16, DH], BF16, tag="ctx")
                        for b in range(B):
                            ex = expp.tile([128, 8, N], BF16, tag="ex")
                            for kb in range(8):
                                gkb = b * 8 + kb
                                ss = pss.tile([128, N], F32, tag="ss")
                                for nq2 in range(2):
                                    nc.tensor.matmul(
                                        ss[:, nq2 * 512:(nq2 + 1) * 512],
                                        lhsT=kT[:, gkb * 128:(gkb + 1) * 128],
                                        rhs=qT[:, b * N + nq2 * 512:
                                               b * N + (nq2 + 1) * 512],
                                        start=True, stop=True)
                                nc.scalar.activation(
                                    out=ex[:, kb, :], in_=ss, func=AF.Exp)
                            for nq2 in range(2):
                                pc = psc.tile([65, 512], F32, tag="pc")
                                for kb in range(8):
                                    nc.tensor.matmul(
                                        pc, lhsT=vaug[:, b * 8 + kb, :],
                                        rhs=ex[:, kb,
                                               nq2 * 512:(nq2 + 1) * 512],
                                        start=(kb == 0), stop=(kb == 7))
                                cts = ctsp.tile([65, 512], BF16, tag="cts")
                                nc.vector.tensor_copy(out=cts, in_=pc)
                                for j in range(4):
                                    ptx = pst.tile([128, 65], BF16, tag="ptx")
                                    nc.tensor.transpose(
                                        ptx, cts[:, j * 128:(j + 1) * 128],
                                        ident[0:65, 0:65])
                                    rec = smallp.tile([128, 1], F32, tag="rec")
                                    nc.vector.reciprocal(out=rec,
                                                         in_=ptx[:, 64:65])
                                    qb = b * 8 + nq2 * 4 + j
                                    nc.vector.tensor_scalar(
                                        out=ctx_sb[:, qb, :],
                                        in0=ptx[:, 0:64], scalar1=rec,
                                        scalar2=None, op0=ALU.mult)
                        nc.sync.dma_start(out=odram, in_=ctx_sb)
    nc.compile()
    return nc


# ---------------------------------------------------------------- launch 2
def build_l2(bdiff: float):
    nc = bacc.Bacc("TRN2", target_bir_lowering=False, debug=False,
                   num_devices=NCORES)
    selfr = nc.dram_tensor("selfr", [RPC, INNER], BF16, kind="ExternalInput").ap()
    crossr = nc.dram_tensor("crossr", [RPC, INNER], BF16, kind="ExternalInput").ap()
    wtr = nc.dram_tensor("wtr", [RPC, INNER], BF16, kind="ExternalInput").ap()
    crossb = nc.dram_tensor("crossb", [N, INNER], BF16, kind="ExternalInput").ap()
    wqg8 = nc.dram_tensor("wqg8", [128, 4, INNER], F8, kind="ExternalInput").ap()
    wkg8 = nc.dram_tensor("wkg8", [128, 4, INNER], F8, kind="ExternalInput").ap()
    wvg8 = nc.dram_tensor("wvg8", [128, 4, INNER], F8, kind="ExternalInput").ap()
    mv2d = nc.dram_tensor("mv2d", [65, 8, 2], BF16, kind="ExternalInput").ap()
    woT = nc.dram_tensor("woT", [128, 4, D], BF16, kind="ExternalInput").ap()
    wf1T = nc.dram_tensor("wf1T", [8, 128, 8, 512], BF16, kind="ExternalInput").ap()
    wf2T = nc.dram_tensor("wf2T", [8, 128, 4, D], BF16, kind="ExternalInput").ap()
    outd = nc.dram_tensor("outd", [RPC, D], F32, kind="ExternalOutput").ap()
    outw = nc.dram_tensor("outw", [RPC, D], F32, kind="ExternalOutput").ap()

    GS = 1.0 / 64.0     # gate weights are shipped fp8 * 64
    KI = INNER // 128   # 4 chunks over INNER
    with tile.TileContext(nc) as tc:
        with tc.tile_pool(name="const", bufs=1) as const, \
             tc.tile_pool(name="wpre", bufs=1) as wpre, \
             tc.tile_pool(name="persist", bufs=1) as persist, \
             tc.tile_pool(name="norm", bufs=4) as norm:
            ident = const.tile([128, 128], BF16)
            make_identity(nc, ident)
            eps_ap = const.tile([128, 1], F32)
            nc.vector.memset(eps_ap, EPS)

            conT8 = persist.tile([128, KI, N], F8)
            sonT8 = persist.tile([128, KI, RPC], F8)
            wtrT = persist.tile([128, KI, RPC], BF16)
            selff = persist.tile([128, 2, INNER], BF16)   # raw self rows
            crossf = persist.tile([128, 2, INNER], BF16)  # raw cross rows

            # activation loads first — the DMA queue drains in order and
            # phase A needs these immediately
            xt4s = []
            for g in range(2):
                xt4 = wpre.tile([128, 4, INNER], BF16, tag=f"xt4{g}",
                                name=f"xt4{g}")
                nc.sync.dma_start(
                    out=xt4,
                    in_=crossb[g * 512:(g + 1) * 512, :].rearrange(
                        "(j p) d -> p j d", p=128))
                xt4s.append(xt4)
            nc.sync.dma_start(
                out=selff, in_=selfr.rearrange("(j p) d -> p j d", p=128))
            nc.sync.dma_start(
                out=crossf, in_=crossr.rearrange("(j p) d -> p j d", p=128))
            wtf = wpre.tile([128, 2, INNER], BF16)
            nc.sync.dma_start(
                out=wtf, in_=wtr.rearrange("(j p) d -> p j d", p=128))

            # then weights, most-urgent first
            wo_sb = wpre.tile([128, KI, D], BF16)
            nc.sync.dma_start(out=wo_sb, in_=woT)
            wk_sb = wpre.tile([128, KI, INNER], F8)
            nc.sync.dma_start(out=wk_sb, in_=wkg8)
            wq_sb = wpre.tile([128, KI, INNER], F8)
            nc.sync.dma_start(out=wq_sb, in_=wqg8)
            wv_sb = wpre.tile([128, KI, INNER], F8)
            nc.sync.dma_start(out=wv_sb, in_=wvg8)
            mv_sb = wpre.tile([65, 8, 2], BF16)
            nc.sync.dma_start(out=mv_sb, in_=mv2d)
            w1all = wpre.tile([128, 8, 8, 512], BF16)
            nc.sync.dma_start(out=w1all,
                              in_=wf1T.rearrange("g p k n -> p g k n"))

            # ---- phase A: LN + transposes
            with tc.tile_pool(name="io", bufs=2) as io, \
                 tc.tile_pool(name="pstr", bufs=4, space="PSUM") as pstr:
                for g in range(2):
                    xt4 = xt4s[g]
                    for j in range(4):
                        tb = g * 4 + j
                        xb = io.tile([128, INNER], BF16, tag="xb")
                        r, nb = _ln_rn(nc, norm, xt4[:, j, :], INNER, eps_ap)
                        if j % 2 == 0:
                            nc.vector.tensor_scalar(
                                out=xb, in0=xt4[:, j, :], scalar1=r,
                                scalar2=nb, op0=ALU.mult, op1=ALU.add)
                        else:
                            nc.scalar.activation(out=xb, in_=xt4[:, j, :],
                                                 func=AF.Identity, bias=nb,
                                                 scale=r)
                        for kc in range(KI):
                            pt = pstr.tile([128, 128], BF16, tag="pt")
                            nc.tensor.transpose(
                                pt, xb[:, kc * 128:(kc + 1) * 128], ident)
                            nc.vector.tensor_copy(
                                out=conT8[:, kc, tb * 128:(tb + 1) * 128],
                                in_=pt)
                for qsb in range(2):
                    sb_ = io.tile([128, INNER], BF16, tag="xb")
                    r, nb = _ln_rn(nc, norm, selff[:, qsb, :], INNER, eps_ap)
                    nc.scalar.activation(out=sb_, in_=selff[:, qsb, :],
                                         func=AF.Identity, bias=nb, scale=r)
                    for kc in range(KI):
                        pt = pstr.tile([128, 128], BF16, tag="pt")
                        nc.tensor.transpose(
                            pt, sb_[:, kc * 128:(kc + 1) * 128], ident)
                        nc.scalar.copy(
                            out=sonT8[:, kc, qsb * 128:(qsb + 1) * 128], in_=pt)
                    for kc in range(KI):
                        pt = pstr.tile([128, 128], BF16, tag="pt")
                        nc.tensor.transpose(
                            pt, wtf[:, qsb, kc * 128:(kc + 1) * 128], ident)
                        nc.scalar.copy(
                            out=wtrT[:, kc, qsb * 128:(qsb + 1) * 128], in_=pt)

            # ---- wt out-projection (independent of the gate chain; emitted
            # early so the PE can fill gaps during gate attention)
            with tc.tile_pool(name="psw", bufs=2, space="PSUM") as psw, \
                 tc.tile_pool(name="outw_p", bufs=4) as outw_p:
                for qsb in range(2):
                    ppw = [psw.tile([128, 512], F32, tag=f"wo{nb_}",
                                    name=f"wo{nb_}") for nb_ in range(2)]
                    for kc in range(KI):
                        for nb_ in range(2):
                            nc.tensor.matmul(
                                ppw[nb_],
                                lhsT=wtrT[:, kc, qsb * 128:(qsb + 1) * 128],
                                rhs=wo_sb[:, kc, nb_ * 512:(nb_ + 1) * 512],
                                start=(kc == 0), stop=(kc == KI - 1))
                    for nb_ in range(2):
                        ow = outw_p.tile([128, 512], F32, tag="ow")
                        nc.scalar.copy(out=ow, in_=ppw[nb_])
                        nc.sync.dma_start(
                            out=outw[qsb * 128:(qsb + 1) * 128,
                                     nb_ * 512:(nb_ + 1) * 512],
                            in_=ow)

            # ---- phase B: gate projections (fp8, DoubleRow over kc pairs)
            DR = mybir.MatmulPerfMode.DoubleRow
            with tc.tile_pool(name="gproj", bufs=1) as gproj:
                kgT = gproj.tile([128, KI, N], BF16)
                qgT = gproj.tile([128, KI, RPC], BF16)
                vaug = gproj.tile([128, 8, H, 65], BF16)
                with tc.tile_pool(name="psb", bufs=2, space="PSUM") as psb:
                    for mo in range(KI):
                        pps = [psb.tile([128, 512], F32, tag=f"gp{nb_}",
                                        name=f"gp{nb_}") for nb_ in range(2)]
                        for kcp in range(2):
                            for nb_ in range(2):
                                nc.tensor.matmul(
                                    pps[nb_],
                                    lhsT=wk_sb[:, 2 * kcp:2 * kcp + 2,
                                               mo * 128:(mo + 1) * 128],
                                    rhs=conT8[:, 2 * kcp:2 * kcp + 2,
                                              nb_ * 512:(nb_ + 1) * 512],
                                    perf_mode=DR,
                                    start=(kcp == 0), stop=(kcp == 1))
                        for nb_ in range(2):
                            nc.vector.tensor_scalar(
                                out=kgT[:, mo, nb_ * 512:(nb_ + 1) * 512],
                                in0=pps[nb_], scalar1=GS, scalar2=None,
                                op0=ALU.mult)
                    for mo in range(KI):
                        ppq = psb.tile([128, RPC], F32, tag="gq")
                        for kcp in range(2):
                            nc.tensor.matmul(
                                ppq,
                                lhsT=wq_sb[:, 2 * kcp:2 * kcp + 2,
                                           mo * 128:(mo + 1) * 128],
                                rhs=sonT8[:, 2 * kcp:2 * kcp + 2, :],
                                perf_mode=DR,
                                start=(kcp == 0), stop=(kcp == 1))
                        nc.vector.tensor_scalar(
                            out=qgT[:, mo, :], in0=ppq, scalar1=GS,
                            scalar2=None, op0=ALU.mult)
                    nc.gpsimd.memset(vaug[:, :, :, 64:65], 1.0)
                    for kb in range(8):
                        pp = psb.tile([128, H, 64], F32, tag="gv")
                        for kcp in range(2):
                            nc.tensor.matmul(
                                pp,
                                lhsT=conT8[:, 2 * kcp:2 * kcp + 2,
                                           kb * 128:(kb + 1) * 128],
                                rhs=wv_sb[:, 2 * kcp:2 * kcp + 2, :],
                                perf_mode=DR,
                                start=(kcp == 0), stop=(kcp == 1))
                        nc.vector.tensor_scalar(
                            out=vaug[:, kb, :, 0:64], in0=pp, scalar1=GS,
                            scalar2=None, op0=ALU.mult)

                # ---- phase C: gate attention, ctx^T + num/den per head
                gctxT = gproj.tile([65, H, RPC], BF16)
                with tc.tile_pool(name="expg", bufs=2) as expg, \
                     tc.tile_pool(name="psg", bufs=2, space="PSUM") as psg, \
                     tc.tile_pool(name="psc", bufs=2, space="PSUM") as psc2:
                    for h in range(H):
                        mo, po = h // 2, (h % 2) * 64
                        ex = expg.tile([128, 8, RPC], BF16, tag="ex")
                        for kq in range(2):
                            ss = psg.tile([128, 4, RPC], F32, tag="ss")
                            for k4 in range(4):
                                kb = kq * 4 + k4
                                nc.tensor.matmul(
                                    ss[:, k4, :],
                                    lhsT=kgT[po:po + 64, mo,
                                             kb * 128:(kb + 1) * 128],
                                    rhs=qgT[po:po + 64, mo, :],
                                    start=True, stop=True)
                            nc.scalar.activation(
                                out=ex[:, kq * 4:(kq + 1) * 4, :], in_=ss,
                                func=AF.Exp)
                        pg = psc2.tile([65, RPC], F32, tag="pg")
                        for kb in range(8):
                            nc.tensor.matmul(
                                pg, lhsT=vaug[:, kb, h, :], rhs=ex[:, kb, :],
                                start=(kb == 0), stop=(kb == 7))
                        nc.vector.tensor_copy(out=gctxT[:, h, :], in_=pg)

                # ---- phase D: mix scalars + mixed + transposes
                mixedT = gproj.tile([128, KI, RPC], BF16)
                with tc.tile_pool(name="mixp", bufs=4) as mixp, \
                     tc.tile_pool(name="pstr3", bufs=4, space="PSUM") as pstr3, \
                     tc.tile_pool(name="psn", bufs=2, space="PSUM") as psn:
                    for qsb in range(2):
                        nd = psn.tile([128, H, 2], F32, tag="nd")
                        for h in range(H):
                            nc.tensor.matmul(
                                nd[:, h, :],
                                lhsT=gctxT[:, h, qsb * 128:(qsb + 1) * 128],
                                rhs=mv_sb[:, h, :], start=True, stop=True)
                        rden = mixp.tile([128, H], F32, tag="rden")
                        nc.vector.reciprocal(out=rden, in_=nd[:, :, 1])
                        tnum = mixp.tile([128, H], F32, tag="tnum")
                        nc.vector.tensor_tensor(out=tnum, in0=nd[:, :, 0],
                                                in1=rden, op=ALU.mult)
                        logit = mixp.tile([128, 1], F32, tag="logit")
                        nc.vector.tensor_reduce(out=logit, in_=tnum,
                                                axis=AX.X, op=ALU.add)
                        mix1 = mixp.tile([128, 1], F32, tag="mix1")
                        nc.scalar.activation(out=mix1, in_=logit,
                                             func=AF.Sigmoid,
                                             bias=float(bdiff), scale=1.0)
                        mix0 = mixp.tile([128, 1], F32, tag="mix0")
                        nc.scalar.activation(out=mix0, in_=logit,
                                             func=AF.Sigmoid,
                                             bias=float(-bdiff), scale=-1.0)
                        t2 = mixp.tile([128, INNER], F32, tag="t2")
                        nc.vector.tensor_scalar_mul(
                            out=t2, in0=crossf[:, qsb, :], scalar1=mix1)
                        mixed_bf = mixp.tile([128, INNER], BF16, tag="mixed")
                        nc.vector.scalar_tensor_tensor(
                            out=mixed_bf, in0=selff[:, qsb, :], scalar=mix0,
                            in1=t2, op0=ALU.mult, op1=ALU.add)
                        for kc in range(KI):
                            pt = pstr3.tile([128, 128], BF16, tag="pt")
                            nc.tensor.transpose(
                                pt, mixed_bf[:, kc * 128:(kc + 1) * 128],
                                ident)
                            nc.scalar.copy(
                                out=mixedT[:, kc, qsb * 128:(qsb + 1) * 128],
                                in_=pt)

                # ---- phase E: delta out-projection
                delta = gproj.tile([128, 2, D], F32)
                with tc.tile_pool(name="pse", bufs=2, space="PSUM") as pse:
                    for qsb in range(2):
                        pps = [pse.tile([128, 512], F32, tag=f"eo{nb_}",
                                        name=f"eo{nb_}")
                               for nb_ in range(2)]
                        for kc in range(KI):
                            for nb_ in range(2):
                                nc.tensor.matmul(
                                    pps[nb_],
                                    lhsT=mixedT[:, kc,
                                                qsb * 128:(qsb + 1) * 128],
                                    rhs=wo_sb[:, kc,
                                              nb_ * 512:(nb_ + 1) * 512],
                                    start=(kc == 0), stop=(kc == KI - 1))
                        for nb_ in range(2):
                            nc.vector.tensor_copy(
                                out=delta[:, qsb,
                                          nb_ * 512:(nb_ + 1) * 512],
                                in_=pps[nb_])

                # ---- phase F: FeedForward
                with tc.tile_pool(name="ffp", bufs=1) as ffp, \
                     tc.tile_pool(name="io2", bufs=3) as io2, \
                     tc.tile_pool(name="psf", bufs=2, space="PSUM") as psf:
                    yT = ffp.tile([128, 8, RPC], BF16)
                    for qsb in range(2):
                        yb = io2.tile([128, D], BF16, tag="yb")
                        r, nb = _ln_rn(nc, norm, delta[:, qsb, :], D, eps_ap)
                        nc.scalar.activation(out=yb, in_=delta[:, qsb, :],
                                             func=AF.Identity, bias=nb,
                                             scale=r)
                        for kc in range(8):
                            pt = psf.tile([128, 128], BF16, tag="pt")
                            nc.tensor.transpose(
                                pt, yb[:, kc * 128:(kc + 1) * 128], ident)
                            nc.scalar.copy(
                                out=yT[:, kc, qsb * 128:(qsb + 1) * 128],
                                in_=pt)
                    h1T = ffp.tile([128, 32, RPC], BF16)
                    with tc.tile_pool(name="psh", bufs=2, space="PSUM") as psh:
                        for mog in range(8):
                            for mo2 in range(2):
                                ph = psh.tile([128, 2, RPC], F32, tag="ph")
                                for mi in range(2):
                                    mo = mo2 * 2 + mi
                                    for kc in range(8):
                                        nc.tensor.matmul(
                                            ph[:, mi, :],
                                            lhsT=w1all[:, mog, kc,
                                                       mo * 128:(mo + 1) * 128],
                                            rhs=yT[:, kc, :],
                                            start=(kc == 0), stop=(kc == 7))
                                nc.scalar.activation(
                                    out=h1T[:, mog * 4 + mo2 * 2:
                                            mog * 4 + mo2 * 2 + 2, :],
                                    in_=ph, func=AF.Gelu)
                    with tc.tile_pool(name="wf2p", bufs=4) as wf2p, \
                         tc.tile_pool(name="psy", bufs=1, space="PSUM") as psy, \
                         tc.tile_pool(name="outd_p", bufs=4) as outd_p:
                        pys = [[psy.tile([128, 512], F32, tag=f"py{q}{n}",
                                         name=f"py{q}{n}")
                                for n in range(2)] for q in range(2)]
                        for g2 in range(8):
                            w2 = wf2p.tile([128, 4, D], BF16, tag="w2")
                            nc.sync.dma_start(out=w2, in_=wf2T[g2])
                            for mo in range(4):
                                mo32 = g2 * 4 + mo
                                for qsb in range(2):
                                    for nb_ in range(2):
                                        nc.tensor.matmul(
                                            pys[qsb][nb_],
                                            lhsT=h1T[:, mo32,
                                                     qsb * 128:(qsb + 1) * 128],
                                            rhs=w2[:, mo,
                                                   nb_ * 512:(nb_ + 1) * 512],
                                            start=(mo32 == 0), stop=(mo32 == 31))
                        for qsb in range(2):
                            for nb_ in range(2):
                                od = outd_p.tile([128, 512], F32, tag="od")
                                nc.vector.tensor_tensor(
                                    out=od, in0=pys[qsb][nb_],
                                    in1=delta[:, qsb, nb_ * 512:(nb_ + 1) * 512],
                                    op=ALU.add)
                                nc.sync.dma_start(
                                    out=outd[qsb * 128:(qsb + 1) * 128,
                                             nb_ * 512:(nb_ + 1) * 512],
                                    in_=od)
    nc.compile()
    return nc


# ---------------------------------------------------------------- host glue
_BUILT = {}
LAST_PROFILE = {}


def _get(key, builder, *args):
    if key not in _BUILT:
        _BUILT[key] = builder(*args)
    return _BUILT[key]


def _bf16(x):
    return np.ascontiguousarray(np.asarray(x).astype(ml_dtypes.bfloat16))


def _fp8(x):
    return np.ascontiguousarray(
        np.clip(np.asarray(x, np.float32), -448, 448).astype(
            ml_dtypes.float8_e4m3fn))


def _shuf(wT, kc):
    """[kc*128, m] -> [128, kc, m] so each SBUF partition row is contiguous."""
    m = wT.shape[1]
    return np.ascontiguousarray(wT.reshape(kc, 128, m).transpose(1, 0, 2))


def _run(nc, in_maps, tag):
    _trace = os.environ.get("KTRACE", "0") == "1"
    res = run_bass_kernel_spmd(nc, in_maps, core_ids=list(range(NCORES)),
                               trace=_trace)
    LAST_PROFILE[f"{tag}_ns"] = res.exec_time_ns
    if res.instructions_and_trace is not None:
        LAST_PROFILE[f"{tag}_trace"] = res.instructions_and_trace[1]
    return res


def kernel(query_feats, kv_feats_wt, nq_w, nq_b, nkv_w, nkv_b, wq_cross,
           wkv_cross, wqkv_self, gn_w, gn_b, mha_in_w, mha_out_w, mix_w,
           mix_b, w_out, ff_ln_w, ff_ln_b, ff_fc1, ff_fc2, ff_gate):
    f = lambda x: np.asarray(x, dtype=np.float32)
    query_feats, kv_feats_wt = f(query_feats), f(kv_feats_wt)
    nq_w, nq_b, nkv_w, nkv_b = f(nq_w), f(nq_b), f(nkv_w), f(nkv_b)
    wq_cross, wkv_cross, wqkv_self = f(wq_cross), f(wkv_cross), f(wqkv_self)
    gn_w, gn_b = f(gn_w), f(gn_b)
    mha_in_w, mha_out_w, mix_w, mix_b = f(mha_in_w), f(mha_out_w), f(mix_w), f(mix_b)
    w_out, ff_ln_w, ff_ln_b = f(w_out), f(ff_ln_w), f(ff_ln_b)
    ff_fc1, ff_fc2, ff_gate = f(ff_fc1), f(ff_fc2), f(ff_gate)

    for b_, nm in ((nq_b, "nq_b"), (nkv_b, "nkv_b"), (gn_b, "gn_b"),
                   (ff_ln_b, "ff_ln_b")):
        assert np.all(b_ == 0.0), f"{nm} != 0 unsupported by this kernel"

    scale = DH ** -0.5
    qf2 = _bf16(query_feats.reshape(T, D))
    kvf2 = _bf16(kv_feats_wt.reshape(T, D))

    # ---------------- launch 0: row-parallel LayerNorm
    nc0 = _get("l0", build_l0)
    in_maps0 = [{"xr": np.concatenate(
        [qf2[c * RPC:(c + 1) * RPC], kvf2[c * RPC:(c + 1) * RPC]], axis=0)}
        for c in range(NCORES)]
    res0 = _run(nc0, in_maps0, "l0")
    qn = np.concatenate(
        [res0.results[c]["xo"][:RPC] for c in range(NCORES)], axis=0)
    kvn = np.concatenate(
        [res0.results[c]["xo"][RPC:] for c in range(NCORES)], axis=0)
    # transpose to [128, kc, T] (pure layout, host-side)
    qnT = np.ascontiguousarray(
        qn.T.reshape(8, 128, T).transpose(1, 0, 2))
    kvnT = np.ascontiguousarray(
        kvn.T.reshape(8, 128, T).transpose(1, 0, 2))

    wq_self = wqkv_self[0:INNER]
    wk_self = wqkv_self[INNER:2 * INNER]
    wv_self = wqkv_self[2 * INNER:3 * INNER]
    wk_cross = wkv_cross[0:INNER]
    wv_cross = wkv_cross[INNER:2 * INNER]

    # ---------------- launch 1
    nc1 = _get("l1", build_l1)
    in_maps1 = []
    for c in range(NCORES):
        s = slice(c * DH, (c + 1) * DH)
        p1 = np.concatenate([
            (wq_cross[s] * nq_w[None, :] * scale).T,
            (wk_self[s] * nq_w[None, :]).T], axis=1)
        p2 = np.concatenate([
            (wv_self[s] * nq_w[None, :]).T,
            (wq_self[s] * nq_w[None, :] * scale).T], axis=1)
        p3 = np.concatenate([
            (wk_cross[s] * nkv_w[None, :]).T,
            (wq_self[s] * nkv_w[None, :] * scale).T], axis=1)
        p4 = np.concatenate([
            (wv_cross[s] * nkv_w[None, :]).T,
            (wk_self[s] * nkv_w[None, :]).T], axis=1)
        p5 = (wv_self[s] * nkv_w[None, :]).T
        in_maps1.append({
            "qnT": qnT, "kvnT": kvnT,
            "p1w": _bf16(_shuf(p1, 8)), "p2w": _bf16(_shuf(p2, 8)),
            "p3w": _bf16(_shuf(p3, 8)), "p4w": _bf16(_shuf(p4, 8)),
            "p5w": _bf16(_shuf(p5, 8)),
        })
    res1 = _run(nc1, in_maps1, "l1")
    # outputs come back [128, 16, 64]; rows t = tb*128 + p
    unblk = lambda a: np.ascontiguousarray(
        a.transpose(1, 0, 2).reshape(T, DH))
    self_out = np.concatenate(
        [unblk(res1.results[c]["self_o"]) for c in range(NCORES)], axis=1)
    cross_out = np.concatenate(
        [unblk(res1.results[c]["cross_o"]) for c in range(NCORES)], axis=1)
    wt_ctx = np.concatenate(
        [unblk(res1.results[c]["wt_o"]) for c in range(NCORES)], axis=1)

    # ---------------- launch 2
    wq_g = mha_in_w[0:INNER]
    wk_g = mha_in_w[INNER:2 * INNER]
    wv_g = mha_in_w[2 * INNER:3 * INNER]
    dmix = mix_w[1] - mix_w[0]
    bdiff = float(mix_b[1] - mix_b[0])
    mvec = (mha_out_w.T @ dmix).reshape(INNER)
    mv2 = np.zeros((65, 8, 2), np.float32)
    for h in range(H):
        mv2[0:64, h, 0] = mvec[h * 64:(h + 1) * 64]
        mv2[64, h, 1] = 1.0
    wqg8 = _fp8(_shuf((wq_g * gn_w[None, :] * scale).T * 64.0, 4))
    wkg8 = _fp8(_shuf((wk_g * gn_w[None, :]).T * 64.0, 4))
    wvg8 = _fp8(_shuf((wv_g * gn_w[None, :]).T * 64.0, 4))
    woT = _bf16(_shuf(w_out.T, 4))
    wf1s = (ff_fc1 * ff_ln_w[None, :]).T          # [D, FF]
    wf1s = wf1s.reshape(8, 128, 8, 512).transpose(2, 1, 0, 3)  # [g,p,kc,n]
    wf2s = (ff_fc2 * float(ff_gate.reshape(-1)[0])).T          # [FF, D]
    wf2s = wf2s.reshape(8, 4, 128, D).transpose(0, 2, 1, 3)    # [g,p,mo,n]

    nc2 = _get(("l2", bdiff), build_l2, bdiff)
    in_maps2 = []
    wf1sb = _bf16(wf1s)
    wf2sb = _bf16(wf2s)
    mv2b = _bf16(mv2)
    for c in range(NCORES):
        g0 = c * RPC
        bb = g0 // N
        in_maps2.append({
            "selfr": self_out[g0:g0 + RPC], "crossr": cross_out[g0:g0 + RPC],
            "wtr": wt_ctx[g0:g0 + RPC],
            "crossb": cross_out[bb * N:(bb + 1) * N],
            "wqg8": wqg8, "wkg8": wkg8, "wvg8": wvg8,
            "mv2d": mv2b, "woT": woT,
            "wf1T": wf1sb, "wf2T": wf2sb,
        })
    res2 = _run(nc2, in_maps2, "l2")
    delta = np.concatenate(
        [res2.results[c]["outd"] for c in range(NCORES)], axis=0)
    wt_out = np.concatenate(
        [res2.results[c]["outw"] for c in range(NCORES)], axis=0)

    return np.stack([delta.reshape(B, N, D),
                     wt_out.reshape(B, N, D)]).astype(np.float32)


# revision 45
# speedup vs baseline: 1.1456x; 1.0171x over previous
"""GatedCrossAttention Trainium2 kernel, v2.

Three SPMD launches with host reshard between (host work is layout only:
slicing, transposes, dtype casts, concatenation — all reference math runs
on device):

  L0 (row-parallel): core c LayerNorms its 256 rows of query_feats and
     kv_feats_wt (stats + apply on device).  Removes the 8x-duplicated
     full-tensor LN of the old design.  Host transposes the gathered
     normalized activations into [128, kc, T] layout for L1.
  L1 (head-parallel): core c owns head c of the three primary attentions
     (cross, self, wt).  Projections consume host-pre-transposed qnT/kvnT
     (no on-device input transposes).  Scores -> one 2-bank exp per key
     block -> value-stationary PV (accumulating ctx^T in PSUM, 8x fewer
     matmul instructions than ex-stationary) -> PE transpose back ->
     per-partition 1/den normalize on DVE.  Outputs normalized per-head
     context slices, bf16.
  L2 (token-parallel): core c owns 256 token rows.  Gate MHA with a
     num/den matmul trick: PV emits gate-ctx^T per head; a tiny [65,2]
     matmul against (mvec_h | e64) yields per-token numerator and
     denominator of the mix logit directly, so the gate context is never
     normalized, copied, or transposed.  Then sigmoid mix, out-projection,
     and the gated FeedForward (batched 2-per-bank gelu).

All LayerNorm affine weights fold into downstream matmul weights
host-side, attention 1/sqrt(d) into q-side weights, ff_gate into fc2,
and mha_out_w + mix_w collapse into one vector (mvec).  Matmuls in bf16
with fp32 PSUM accumulation; softmax skips max subtraction (logits are
small, overflow impossible).
"""
import os
import sys
sys.path.insert(0, '/opt/trn_rl_repo')

import numpy as np
import ml_dtypes

import concourse.bass as bass
import concourse.bacc as bacc
import concourse.tile as tile
import concourse.mybir as mybir
from concourse.bass_utils import run_bass_kernel_spmd
from concourse.masks import make_identity

F32 = mybir.dt.float32
BF16 = mybir.dt.bfloat16
F8 = mybir.dt.float8e4
AF = mybir.ActivationFunctionType
ALU = mybir.AluOpType
AX = mybir.AxisListType

B, N, D = 2, 1024, 1024
H, DH = 8, 64
INNER = 512
FF = 4096
T = B * N            # 2048 flattened tokens
EPS = 1e-5
NCORES = 8
RPC = T // NCORES    # 256 rows per core (L0 / L2)


# ---------------------------------------------------------------- helpers
def _ln_rn(nc, norm, xt, ncols, eps_ap):
    """Stats for LayerNorm of xt [128, ncols]: returns (r, nb) f32 [128,1]
    with y = x*r + nb."""
    nsub = ncols // 512
    st = norm.tile([128, nsub, 6], F32, tag="st")
    for s in range(nsub):
        nc.vector.bn_stats(out=st[:, s, :], in_=xt[:, s * 512:(s + 1) * 512])
    mv = norm.tile([128, 2], F32, tag="mv")
    nc.vector.bn_aggr(out=mv, in_=st)
    sd = norm.tile([128, 1], F32, tag="sd")
    nc.scalar.activation(out=sd, in_=mv[:, 1:2], func=AF.Sqrt, bias=eps_ap)
    r = norm.tile([128, 1], F32, tag="r")
    nc.vector.reciprocal(out=r, in_=sd)
    nb = norm.tile([128, 1], F32, tag="nb")
    nc.vector.tensor_scalar(out=nb, in0=mv[:, 0:1], scalar1=r, scalar2=-1.0,
                            op0=ALU.mult, op1=ALU.mult)
    return r, nb


# ---------------------------------------------------------------- launch 0
def build_l0():
    nc = bacc.Bacc("TRN2", target_bir_lowering=False, debug=False,
                   num_devices=NCORES)
    xr = nc.dram_tensor("xr", [2 * RPC, D], BF16, kind="ExternalInput").ap()
    xo = nc.dram_tensor("xo", [2 * RPC, D], BF16, kind="ExternalOutput").ap()
    with tile.TileContext(nc) as tc:
        with tc.tile_pool(name="io", bufs=1) as io, \
             tc.tile_pool(name="norm", bufs=1) as norm:
            eps_ap = norm.tile([128, 1], F32, tag="eps")
            nc.vector.memset(eps_ap, EPS)
            xt = io.tile([128, 4, D], BF16, tag="xt")
            nc.sync.dma_start(
                out=xt, in_=xr.rearrange("(j p) d -> p j d", p=128))
            ot = io.tile([128, 4, D], BF16, tag="ot")
            # batched stats for all 4 row tiles: one sqrt/recip round trip
            st = norm.tile([128, 4, 2, 6], F32, tag="st")
            for j in range(4):
                for s in range(2):
                    nc.vector.bn_stats(out=st[:, j, s, :],
                                       in_=xt[:, j, s * 512:(s + 1) * 512])
            mv4 = norm.tile([128, 4, 2], F32, tag="mv4")
            for j in range(4):
                nc.vector.bn_aggr(out=mv4[:, j, :], in_=st[:, j, :, :])
            sd4 = norm.tile([128, 4], F32, tag="sd4")
            nc.scalar.activation(out=sd4, in_=mv4[:, :, 1], func=AF.Sqrt,
                                 bias=eps_ap)
            r4 = norm.tile([128, 4], F32, tag="r4")
            nc.vector.reciprocal(out=r4, in_=sd4)
            nb4 = norm.tile([128, 4], F32, tag="nb4")
            nc.vector.tensor_tensor(out=nb4, in0=mv4[:, :, 0], in1=r4,
                                    op=ALU.mult)
            nc.vector.tensor_scalar_mul(out=nb4, in0=nb4, scalar1=-1.0)
            for j in range(4):
                if j % 2 == 0:
                    nc.vector.tensor_scalar(
                        out=ot[:, j, :], in0=xt[:, j, :],
                        scalar1=r4[:, j:j + 1], scalar2=nb4[:, j:j + 1],
                        op0=ALU.mult, op1=ALU.add)
                else:
                    nc.scalar.activation(out=ot[:, j, :], in_=xt[:, j, :],
                                         func=AF.Identity,
                                         bias=nb4[:, j:j + 1],
                                         scale=r4[:, j:j + 1])
            nc.sync.dma_start(
                out=xo.rearrange("(j p) d -> p j d", p=128), in_=ot)
    nc.compile()
    return nc


# ---------------------------------------------------------------- launch 1
def build_l1():
    nc = bacc.Bacc("TRN2", target_bir_lowering=False, debug=False,
                   num_devices=NCORES)
    qnT = nc.dram_tensor("qnT", [128, 8, T], BF16, kind="ExternalInput").ap()
    kvnT = nc.dram_tensor("kvnT", [128, 8, T], BF16, kind="ExternalInput").ap()
    pw = nc.dram_tensor("pw", [128, 8, 576], BF16, kind="ExternalInput").ap()
    # outputs in [p, tblock, d] layout; host reassembles rows
    self_o = nc.dram_tensor("self_o", [128, 16, DH], BF16,
                            kind="ExternalOutput").ap()
    cross_o = nc.dram_tensor("cross_o", [128, 16, DH], BF16,
                             kind="ExternalOutput").ap()
    wt_o = nc.dram_tensor("wt_o", [128, 16, DH], BF16,
                          kind="ExternalOutput").ap()

    KC = D // 128    # 8 channel chunks

    with tile.TileContext(nc) as tc:
        with tc.tile_pool(name="const", bufs=1) as const, \
             tc.tile_pool(name="wein", bufs=1) as wein, \
             tc.tile_pool(name="xin", bufs=1) as xin, \
             tc.tile_pool(name="projT", bufs=1) as projT:
            ident = const.tile([128, 128], BF16)
            make_identity(nc, ident)

            # one weight DMA first (the DMA queue drains in issue order and
            # every dma_start costs ~1.2us of serialized issue time)
            wall = wein.tile([128, KC, 576], BF16)
            nc.sync.dma_start(out=wall, in_=pw)
            wsbs = [wall[:, :, 0:128], wall[:, :, 128:256],
                    wall[:, :, 256:384], wall[:, :, 384:512],
                    wall[:, :, 512:576]]

            qn_sb = xin.tile([128, KC, T], BF16)
            kvn_sb = xin.tile([128, KC, T], BF16)
            # half-column loads so projections start before the full
            # activations arrive
            for th in range(2):
                cs = slice(th * 1024, (th + 1) * 1024)
                nc.sync.dma_start(out=qn_sb[:, :, cs], in_=qnT[:, :, cs])
                nc.sync.dma_start(out=kvn_sb[:, :, cs], in_=kvnT[:, :, cs])

            p1T = projT.tile([128, T], BF16)   # [q_c | k_s]
            p2T = projT.tile([128, T], BF16)   # [v_s | q_s]
            p3T = projT.tile([128, T], BF16)   # [k_c | q_wt]
            p4T = projT.tile([128, T], BF16)   # [v_c | k_wt]
            p5T = projT.tile([64, T], BF16)    # v_wt

            # ---- projections (weight stationary held over 4 psum banks)
            with tc.tile_pool(name="psproj", bufs=2, space="PSUM") as psp:
                for woff, xsb, dst, mwid in (
                        (0, qn_sb, p1T, 128),
                        (128, qn_sb, p2T, 128),
                        (256, kvn_sb, p3T, 128),
                        (384, kvn_sb, p4T, 128),
                        (512, kvn_sb, p5T, 64)):
                    pps = [psp.tile([128, 512], F32, tag=f"pp{nb_}",
                                    name=f"pp{nb_}") for nb_ in range(4)]
                    for kc in range(KC):
                        for nb_ in range(4):
                            nc.tensor.matmul(
                                pps[nb_][:mwid, :],
                                lhsT=wall[:, kc, woff:woff + mwid],
                                rhs=xsb[:, kc, nb_ * 512:(nb_ + 1) * 512],
                                start=(kc == 0), stop=(kc == KC - 1))
                    for nb_ in range(4):
                        nc.vector.tensor_copy(
                            out=dst[:, nb_ * 512:(nb_ + 1) * 512],
                            in_=pps[nb_][:mwid, :])

            # ---- v transposes + ones column
            with tc.tile_pool(name="vaugp", bufs=1) as vaugp:
                vaug_c = vaugp.tile([128, 16, 65], BF16)
                vaug_s = vaugp.tile([128, 16, 65], BF16)
                vaug_w = vaugp.tile([128, 16, 65], BF16)
                with tc.tile_pool(name="pstr2", bufs=4,
                                  space="PSUM") as pstr2:
                    for srcT, vaug in ((p4T[0:64, :], vaug_c),
                                       (p2T[0:64, :], vaug_s),
                                       (p5T[0:64, :], vaug_w)):
                        nc.gpsimd.memset(vaug[:, :, 64:65], 1.0)
                        for kb in range(16):
                            pt2 = pstr2.tile([128, 64], BF16, tag="pt2")
                            nc.tensor.transpose(
                                pt2, srcT[:, kb * 128:(kb + 1) * 128],
                                ident[0:64, 0:64])
                            nc.vector.tensor_copy(out=vaug[:, kb, 0:64],
                                                  in_=pt2)

                # ---- attentions
                specs = (
                    (p1T[0:64, :], p3T[0:64, :], vaug_c, cross_o),
                    (p2T[64:128, :], p1T[64:128, :], vaug_s, self_o),
                    (p3T[64:128, :], p4T[64:128, :], vaug_w, wt_o),
                )
                with tc.tile_pool(name="expp", bufs=2) as expp, \
                     tc.tile_pool(name="ctsp", bufs=2) as ctsp, \
                     tc.tile_pool(name="smallp", bufs=8) as smallp, \
                     tc.tile_pool(name="ctxp", bufs=2) as ctxp, \
                     tc.tile_pool(name="pss", bufs=2, space="PSUM") as pss, \
                     tc.tile_pool(name="psc", bufs=2, space="PSUM") as psc, \
                     tc.tile_pool(name="pst", bufs=2, space="PSUM") as pst:
                    for qT, kT, vaug, odram in specs:
                        ctx_sb = ctxp.tile([128, 16, DH], BF16, tag="ctx")
                        for b in range(B):
                            ex = expp.tile([128, 8, N], BF16, tag="ex")
                            for kb in range(8):
                                gkb = b * 8 + kb
                                ss = pss.tile([128, N], F32, tag="ss")
                                for nq2 in range(2):
                                    nc.tensor.matmul(
                                        ss[:, nq2 * 512:(nq2 + 1) * 512],
                                        lhsT=kT[:, gkb * 128:(gkb + 1) * 128],
                                        rhs=qT[:, b * N + nq2 * 512:
                                               b * N + (nq2 + 1) * 512],
                                        start=True, stop=True)
                                nc.scalar.activation(
                                    out=ex[:, kb, :], in_=ss, func=AF.Exp)
                            for nq2 in range(2):
                                pc = psc.tile([65, 512], F32, tag="pc")
                                for kb in range(8):
                                    nc.tensor.matmul(
                                        pc, lhsT=vaug[:, b * 8 + kb, :],
                                        rhs=ex[:, kb,
                                               nq2 * 512:(nq2 + 1) * 512],
                                        start=(kb == 0), stop=(kb == 7))
                                cts = ctsp.tile([65, 512], BF16, tag="cts")
                                nc.vector.tensor_copy(out=cts, in_=pc)
                                for j in range(4):
                                    ptx = pst.tile([128, 65], BF16, tag="ptx")
                                    nc.tensor.transpose(
                                        ptx, cts[:, j * 128:(j + 1) * 128],
                                        ident[0:65, 0:65])
                                    rec = smallp.tile([128, 1], F32, tag="rec")
                                    nc.vector.reciprocal(out=rec,
                                                         in_=ptx[:, 64:65])
                                    qb = b * 8 + nq2 * 4 + j
                                    nc.vector.tensor_scalar(
                                        out=ctx_sb[:, qb, :],
                                        in0=ptx[:, 0:64], scalar1=rec,
                                        scalar2=None, op0=ALU.mult)
                        nc.sync.dma_start(out=odram, in_=ctx_sb)
    nc.compile()
    return nc


# ---------------------------------------------------------------- launch 2
def build_l2(bdiff: float):
    nc = bacc.Bacc("TRN2", target_bir_lowering=False, debug=False,
                   num_devices=NCORES)
    rows3d = nc.dram_tensor("rows3d", [3 * RPC, INNER], BF16,
                            kind="ExternalInput").ap()
    crossb = nc.dram_tensor("crossb", [N, INNER], BF16, kind="ExternalInput").ap()
    gw8 = nc.dram_tensor("gw8", [128, 4, 3 * INNER], F8,
                         kind="ExternalInput").ap()
    mv2d = nc.dram_tensor("mv2d", [65, 8, 2], BF16, kind="ExternalInput").ap()
    woT = nc.dram_tensor("woT", [128, 4, D], BF16, kind="ExternalInput").ap()
    wf1T = nc.dram_tensor("wf1T", [8, 128, 8, 512], BF16, kind="ExternalInput").ap()
    wf2T = nc.dram_tensor("wf2T", [8, 128, 4, D], BF16, kind="ExternalInput").ap()
    outd = nc.dram_tensor("outd", [RPC, D], F32, kind="ExternalOutput").ap()
    outw = nc.dram_tensor("outw", [RPC, D], F32, kind="ExternalOutput").ap()

    GS = 1.0 / 64.0     # gate weights are shipped fp8 * 64
    KI = INNER // 128   # 4 chunks over INNER
    with tile.TileContext(nc) as tc:
        with tc.tile_pool(name="const", bufs=1) as const, \
             tc.tile_pool(name="wpre", bufs=1) as wpre, \
             tc.tile_pool(name="persist", bufs=1) as persist, \
             tc.tile_pool(name="norm", bufs=4) as norm:
            ident = const.tile([128, 128], BF16)
            make_identity(nc, ident)
            eps_ap = const.tile([128, 1], F32)
            nc.vector.memset(eps_ap, EPS)

            conT8 = persist.tile([128, KI, N], F8)
            sonT8 = persist.tile([128, KI, RPC], F8)
            wtrT = persist.tile([128, KI, RPC], BF16)

            # activation loads first — the DMA queue drains in order and
            # phase A needs these immediately
            xt8 = wpre.tile([128, 8, INNER], BF16)
            nc.sync.dma_start(
                out=xt8, in_=crossb.rearrange("(j p) d -> p j d", p=128))
            rows3 = wpre.tile([128, 6, INNER], BF16)
            nc.sync.dma_start(
                out=rows3, in_=rows3d.rearrange("(j p) d -> p j d", p=128))
            selff = rows3[:, 0:2, :]    # raw self rows
            crossf = rows3[:, 2:4, :]   # raw cross rows
            wtf = rows3[:, 4:6, :]      # raw wt rows

            # then weights, most-urgent first
            wo_sb = wpre.tile([128, KI, D], BF16)
            nc.sync.dma_start(out=wo_sb, in_=woT)
            gwall = wpre.tile([128, KI, 3 * INNER], F8)
            nc.sync.dma_start(out=gwall, in_=gw8)
            wk_sb = gwall[:, :, 0:INNER]
            wq_sb = gwall[:, :, INNER:2 * INNER]
            wv_sb = gwall[:, :, 2 * INNER:3 * INNER]
            mv_sb = wpre.tile([65, 8, 2], BF16)
            nc.sync.dma_start(out=mv_sb, in_=mv2d)
            w1all = wpre.tile([128, 8, 8, 512], BF16)
            nc.sync.dma_start(out=w1all,
                              in_=wf1T.rearrange("g p k n -> p g k n"))

            # ---- phase A: LN + transposes
            with tc.tile_pool(name="io", bufs=2) as io, \
                 tc.tile_pool(name="pstr", bufs=4, space="PSUM") as pstr:
                for tb in range(8):
                        j = tb
                        xb = io.tile([128, INNER], BF16, tag="xb")
                        r, nb = _ln_rn(nc, norm, xt8[:, j, :], INNER, eps_ap)
                        if j % 2 == 0:
                            nc.vector.tensor_scalar(
                                out=xb, in0=xt8[:, j, :], scalar1=r,
                                scalar2=nb, op0=ALU.mult, op1=ALU.add)
                        else:
                            nc.scalar.activation(out=xb, in_=xt8[:, j, :],
                                                 func=AF.Identity, bias=nb,
                                                 scale=r)
                        for kc in range(KI):
                            pt = pstr.tile([128, 128], BF16, tag="pt")
                            nc.tensor.transpose(
                                pt, xb[:, kc * 128:(kc + 1) * 128], ident)
                            nc.vector.tensor_copy(
                                out=conT8[:, kc, tb * 128:(tb + 1) * 128],
                                in_=pt)
                for qsb in range(2):
                    sb_ = io.tile([128, INNER], BF16, tag="xb")
                    r, nb = _ln_rn(nc, norm, selff[:, qsb, :], INNER, eps_ap)
                    nc.scalar.activation(out=sb_, in_=selff[:, qsb, :],
                                         func=AF.Identity, bias=nb, scale=r)
                    for kc in range(KI):
                        pt = pstr.tile([128, 128], BF16, tag="pt")
                        nc.tensor.transpose(
                            pt, sb_[:, kc * 128:(kc + 1) * 128], ident)
                        nc.scalar.copy(
                            out=sonT8[:, kc, qsb * 128:(qsb + 1) * 128], in_=pt)
                    for kc in range(KI):
                        pt = pstr.tile([128, 128], BF16, tag="pt")
                        nc.tensor.transpose(
                            pt, wtf[:, qsb, kc * 128:(kc + 1) * 128], ident)
                        nc.scalar.copy(
                            out=wtrT[:, kc, qsb * 128:(qsb + 1) * 128], in_=pt)

            # ---- wt out-projection (independent of the gate chain; emitted
            # early so the PE can fill gaps during gate attention)
            with tc.tile_pool(name="psw", bufs=2, space="PSUM") as psw, \
                 tc.tile_pool(name="outw_p", bufs=4) as outw_p:
                for qsb in range(2):
                    ppw = [psw.tile([128, 512], F32, tag=f"wo{nb_}",
                                    name=f"wo{nb_}") for nb_ in range(2)]
                    for kc in range(KI):
                        for nb_ in range(2):
                            nc.tensor.matmul(
                                ppw[nb_],
                                lhsT=wtrT[:, kc, qsb * 128:(qsb + 1) * 128],
                                rhs=wo_sb[:, kc, nb_ * 512:(nb_ + 1) * 512],
                                start=(kc == 0), stop=(kc == KI - 1))
                    ow = outw_p.tile([128, D], F32, tag="ow")
                    for nb_ in range(2):
                        nc.scalar.copy(out=ow[:, nb_ * 512:(nb_ + 1) * 512],
                                       in_=ppw[nb_])
                    nc.sync.dma_start(
                        out=outw[qsb * 128:(qsb + 1) * 128, :], in_=ow)

            # ---- phase B: gate projections (fp8, DoubleRow over kc pairs)
            DR = mybir.MatmulPerfMode.DoubleRow
            with tc.tile_pool(name="gproj", bufs=1) as gproj:
                kgT = gproj.tile([128, KI, N], BF16)
                qgT = gproj.tile([128, KI, RPC], BF16)
                vaug = gproj.tile([128, 8, H, 65], BF16)
                with tc.tile_pool(name="psb", bufs=2, space="PSUM") as psb:
                    for mo in range(KI):
                        pps = [psb.tile([128, 512], F32, tag=f"gp{nb_}",
                                        name=f"gp{nb_}") for nb_ in range(2)]
                        for kcp in range(2):
                            for nb_ in range(2):
                                nc.tensor.matmul(
                                    pps[nb_],
                                    lhsT=wk_sb[:, 2 * kcp:2 * kcp + 2,
                                               mo * 128:(mo + 1) * 128],
                                    rhs=conT8[:, 2 * kcp:2 * kcp + 2,
                                              nb_ * 512:(nb_ + 1) * 512],
                                    perf_mode=DR,
                                    start=(kcp == 0), stop=(kcp == 1))
                        for nb_ in range(2):
                            nc.vector.tensor_scalar(
                                out=kgT[:, mo, nb_ * 512:(nb_ + 1) * 512],
                                in0=pps[nb_], scalar1=GS, scalar2=None,
                                op0=ALU.mult)
                    for mo in range(KI):
                        ppq = psb.tile([128, RPC], F32, tag="gq")
                        for kcp in range(2):
                            nc.tensor.matmul(
                                ppq,
                                lhsT=wq_sb[:, 2 * kcp:2 * kcp + 2,
                                           mo * 128:(mo + 1) * 128],
                                rhs=sonT8[:, 2 * kcp:2 * kcp + 2, :],
                                perf_mode=DR,
                                start=(kcp == 0), stop=(kcp == 1))
                        nc.vector.tensor_scalar(
                            out=qgT[:, mo, :], in0=ppq, scalar1=GS,
                            scalar2=None, op0=ALU.mult)
                    nc.gpsimd.memset(vaug[:, :, :, 64:65], 1.0)
                    for kb in range(8):
                        pp = psb.tile([128, H, 64], F32, tag="gv")
                        for kcp in range(2):
                            nc.tensor.matmul(
                                pp,
                                lhsT=conT8[:, 2 * kcp:2 * kcp + 2,
                                           kb * 128:(kb + 1) * 128],
                                rhs=wv_sb[:, 2 * kcp:2 * kcp + 2, :],
                                perf_mode=DR,
                                start=(kcp == 0), stop=(kcp == 1))
                        nc.vector.tensor_scalar(
                            out=vaug[:, kb, :, 0:64], in0=pp, scalar1=GS,
                            scalar2=None, op0=ALU.mult)

                # ---- phase C: gate attention, ctx^T + num/den per head
                gctxT = gproj.tile([65, H, RPC], BF16)
                with tc.tile_pool(name="expg", bufs=2) as expg, \
                     tc.tile_pool(name="psg", bufs=2, space="PSUM") as psg, \
                     tc.tile_pool(name="psc", bufs=2, space="PSUM") as psc2:
                    for h in range(H):
                        mo, po = h // 2, (h % 2) * 64
                        ex = expg.tile([128, 8, RPC], BF16, tag="ex")
                        for kq in range(2):
                            ss = psg.tile([128, 4, RPC], F32, tag="ss")
                            for k4 in range(4):
                                kb = kq * 4 + k4
                                nc.tensor.matmul(
                                    ss[:, k4, :],
                                    lhsT=kgT[po:po + 64, mo,
                                             kb * 128:(kb + 1) * 128],
                                    rhs=qgT[po:po + 64, mo, :],
                                    start=True, stop=True)
                            nc.scalar.activation(
                                out=ex[:, kq * 4:(kq + 1) * 4, :], in_=ss,
                                func=AF.Exp)
                        pg = psc2.tile([65, RPC], F32, tag="pg")
                        for kb in range(8):
                            nc.tensor.matmul(
                                pg, lhsT=vaug[:, kb, h, :], rhs=ex[:, kb, :],
                                start=(kb == 0), stop=(kb == 7))
                        nc.vector.tensor_copy(out=gctxT[:, h, :], in_=pg)

                # ---- phase D: mix scalars + mixed + transposes
                mixedT = gproj.tile([128, KI, RPC], BF16)
                with tc.tile_pool(name="mixp", bufs=4) as mixp, \
                     tc.tile_pool(name="pstr3", bufs=4, space="PSUM") as pstr3, \
                     tc.tile_pool(name="psn", bufs=2, space="PSUM") as psn:
                    for qsb in range(2):
                        nd = psn.tile([128, H, 2], F32, tag="nd")
                        for h in range(H):
                            nc.tensor.matmul(
                                nd[:, h, :],
                                lhsT=gctxT[:, h, qsb * 128:(qsb + 1) * 128],
                                rhs=mv_sb[:, h, :], start=True, stop=True)
                        rden = mixp.tile([128, H], F32, tag="rden")
                        nc.vector.reciprocal(out=rden, in_=nd[:, :, 1])
                        tnum = mixp.tile([128, H], F32, tag="tnum")
                        nc.vector.tensor_tensor(out=tnum, in0=nd[:, :, 0],
                                                in1=rden, op=ALU.mult)
                        logit = mixp.tile([128, 1], F32, tag="logit")
                        nc.vector.tensor_reduce(out=logit, in_=tnum,
                                                axis=AX.X, op=ALU.add)
                        mix1 = mixp.tile([128, 1], F32, tag="mix1")
                        nc.scalar.activation(out=mix1, in_=logit,
                                             func=AF.Sigmoid,
                                             bias=float(bdiff), scale=1.0)
                        mix0 = mixp.tile([128, 1], F32, tag="mix0")
                        nc.scalar.activation(out=mix0, in_=logit,
                                             func=AF.Sigmoid,
                                             bias=float(-bdiff), scale=-1.0)
                        t2 = mixp.tile([128, INNER], F32, tag="t2")
                        nc.vector.tensor_scalar_mul(
                            out=t2, in0=crossf[:, qsb, :], scalar1=mix1)
                        mixed_bf = mixp.tile([128, INNER], BF16, tag="mixed")
                        nc.vector.scalar_tensor_tensor(
                            out=mixed_bf, in0=selff[:, qsb, :], scalar=mix0,
                            in1=t2, op0=ALU.mult, op1=ALU.add)
                        for kc in range(KI):
                            pt = pstr3.tile([128, 128], BF16, tag="pt")
                            nc.tensor.transpose(
                                pt, mixed_bf[:, kc * 128:(kc + 1) * 128],
                                ident)
                            nc.scalar.copy(
                                out=mixedT[:, kc, qsb * 128:(qsb + 1) * 128],
                                in_=pt)

                # ---- phase E: delta out-projection
                delta = gproj.tile([128, 2, D], F32)
                with tc.tile_pool(name="pse", bufs=2, space="PSUM") as pse:
                    for qsb in range(2):
                        pps = [pse.tile([128, 512], F32, tag=f"eo{nb_}",
                                        name=f"eo{nb_}")
                               for nb_ in range(2)]
                        for kc in range(KI):
                            for nb_ in range(2):
                                nc.tensor.matmul(
                                    pps[nb_],
                                    lhsT=mixedT[:, kc,
                                                qsb * 128:(qsb + 1) * 128],
                                    rhs=wo_sb[:, kc,
                                              nb_ * 512:(nb_ + 1) * 512],
                                    start=(kc == 0), stop=(kc == KI - 1))
                        for nb_ in range(2):
                            nc.vector.tensor_copy(
                                out=delta[:, qsb,
                                          nb_ * 512:(nb_ + 1) * 512],
                                in_=pps[nb_])

                # ---- phase F: FeedForward
                with tc.tile_pool(name="ffp", bufs=1) as ffp, \
                     tc.tile_pool(name="io2", bufs=3) as io2, \
                     tc.tile_pool(name="psf", bufs=2, space="PSUM") as psf:
                    yT = ffp.tile([128, 8, RPC], BF16)
                    for qsb in range(2):
                        yb = io2.tile([128, D], BF16, tag="yb")
                        r, nb = _ln_rn(nc, norm, delta[:, qsb, :], D, eps_ap)
                        nc.scalar.activation(out=yb, in_=delta[:, qsb, :],
                                             func=AF.Identity, bias=nb,
                                             scale=r)
                        for kc in range(8):
                            pt = psf.tile([128, 128], BF16, tag="pt")
                            nc.tensor.transpose(
                                pt, yb[:, kc * 128:(kc + 1) * 128], ident)
                            nc.scalar.copy(
                                out=yT[:, kc, qsb * 128:(qsb + 1) * 128],
                                in_=pt)
                    h1T = ffp.tile([128, 32, RPC], BF16)
                    with tc.tile_pool(name="psh", bufs=2, space="PSUM") as psh:
                        for mog in range(8):
                            for mo2 in range(2):
                                ph = psh.tile([128, 2, RPC], F32, tag="ph")
                                for mi in range(2):
                                    mo = mo2 * 2 + mi
                                    for kc in range(8):
                                        nc.tensor.matmul(
                                            ph[:, mi, :],
                                            lhsT=w1all[:, mog, kc,
                                                       mo * 128:(mo + 1) * 128],
                                            rhs=yT[:, kc, :],
                                            start=(kc == 0), stop=(kc == 7))
                                nc.scalar.activation(
                                    out=h1T[:, mog * 4 + mo2 * 2:
                                            mog * 4 + mo2 * 2 + 2, :],
                                    in_=ph, func=AF.Gelu)
                    with tc.tile_pool(name="wf2p", bufs=4) as wf2p, \
                         tc.tile_pool(name="psy", bufs=1, space="PSUM") as psy, \
                         tc.tile_pool(name="outd_p", bufs=4) as outd_p:
                        pys = [[psy.tile([128, 512], F32, tag=f"py{q}{n}",
                                         name=f"py{q}{n}")
                                for n in range(2)] for q in range(2)]
                        for g2 in range(8):
                            w2 = wf2p.tile([128, 4, D], BF16, tag="w2")
                            nc.sync.dma_start(out=w2, in_=wf2T[g2])
                            for mo in range(4):
                                mo32 = g2 * 4 + mo
                                for qsb in range(2):
                                    for nb_ in range(2):
                                        nc.tensor.matmul(
                                            pys[qsb][nb_],
                                            lhsT=h1T[:, mo32,
                                                     qsb * 128:(qsb + 1) * 128],
                                            rhs=w2[:, mo,
                                                   nb_ * 512:(nb_ + 1) * 512],
                                            start=(mo32 == 0), stop=(mo32 == 31))
                        for qsb in range(2):
                            for nb_ in range(2):
                                od = outd_p.tile([128, 512], F32, tag="od")
                                nc.vector.tensor_tensor(
                                    out=od, in0=pys[qsb][nb_],
                                    in1=delta[:, qsb, nb_ * 512:(nb_ + 1) * 512],
                                    op=ALU.add)
                                nc.sync.dma_start(
                                    out=outd[qsb * 128:(qsb + 1) * 128,
                                             nb_ * 512:(nb_ + 1) * 512],
                                    in_=od)
    nc.compile()
    return nc


# ---------------------------------------------------------------- host glue
_BUILT = {}
LAST_PROFILE = {}


def _get(key, builder, *args):
    if key not in _BUILT:
        _BUILT[key] = builder(*args)
    return _BUILT[key]


def _bf16(x):
    return np.ascontiguousarray(np.asarray(x).astype(ml_dtypes.bfloat16))


def _fp8(x):
    return np.ascontiguousarray(
        np.clip(np.asarray(x, np.float32), -448, 448).astype(
            ml_dtypes.float8_e4m3fn))


def _shuf(wT, kc):
    """[kc*128, m] -> [128, kc, m] so each SBUF partition row is contiguous."""
    m = wT.shape[1]
    return np.ascontiguousarray(wT.reshape(kc, 128, m).transpose(1, 0, 2))


def _run(nc, in_maps, tag):
    _trace = os.environ.get("KTRACE", "0") == "1"
    res = run_bass_kernel_spmd(nc, in_maps, core_ids=list(range(NCORES)),
                               trace=_trace)
    LAST_PROFILE[f"{tag}_ns"] = res.exec_time_ns
    if res.instructions_and_trace is not None:
        LAST_PROFILE[f"{tag}_trace"] = res.instructions_and_trace[1]
    return res


def kernel(query_feats, kv_feats_wt, nq_w, nq_b, nkv_w, nkv_b, wq_cross,
           wkv_cross, wqkv_self, gn_w, gn_b, mha_in_w, mha_out_w, mix_w,
           mix_b, w_out, ff_ln_w, ff_ln_b, ff_fc1, ff_fc2, ff_gate):
    f = lambda x: np.asarray(x, dtype=np.float32)
    query_feats, kv_feats_wt = f(query_feats), f(kv_feats_wt)
    nq_w, nq_b, nkv_w, nkv_b = f(nq_w), f(nq_b), f(nkv_w), f(nkv_b)
    wq_cross, wkv_cross, wqkv_self = f(wq_cross), f(wkv_cross), f(wqkv_self)
    gn_w, gn_b = f(gn_w), f(gn_b)
    mha_in_w, mha_out_w, mix_w, mix_b = f(mha_in_w), f(mha_out_w), f(mix_w), f(mix_b)
    w_out, ff_ln_w, ff_ln_b = f(w_out), f(ff_ln_w), f(ff_ln_b)
    ff_fc1, ff_fc2, ff_gate = f(ff_fc1), f(ff_fc2), f(ff_gate)

    for b_, nm in ((nq_b, "nq_b"), (nkv_b, "nkv_b"), (gn_b, "gn_b"),
                   (ff_ln_b, "ff_ln_b")):
        assert np.all(b_ == 0.0), f"{nm} != 0 unsupported by this kernel"

    scale = DH ** -0.5
    qf2 = _bf16(query_feats.reshape(T, D))
    kvf2 = _bf16(kv_feats_wt.reshape(T, D))

    # ---------------- launch 0: row-parallel LayerNorm
    nc0 = _get("l0", build_l0)
    in_maps0 = [{"xr": np.concatenate(
        [qf2[c * RPC:(c + 1) * RPC], kvf2[c * RPC:(c + 1) * RPC]], axis=0)}
        for c in range(NCORES)]
    res0 = _run(nc0, in_maps0, "l0")
    qn = np.concatenate(
        [res0.results[c]["xo"][:RPC] for c in range(NCORES)], axis=0)
    kvn = np.concatenate(
        [res0.results[c]["xo"][RPC:] for c in range(NCORES)], axis=0)
    # transpose to [128, kc, T] (pure layout, host-side)
    qnT = np.ascontiguousarray(
        qn.T.reshape(8, 128, T).transpose(1, 0, 2))
    kvnT = np.ascontiguousarray(
        kvn.T.reshape(8, 128, T).transpose(1, 0, 2))

    wq_self = wqkv_self[0:INNER]
    wk_self = wqkv_self[INNER:2 * INNER]
    wv_self = wqkv_self[2 * INNER:3 * INNER]
    wk_cross = wkv_cross[0:INNER]
    wv_cross = wkv_cross[INNER:2 * INNER]

    # ---------------- launch 1
    nc1 = _get("l1", build_l1)
    in_maps1 = []
    for c in range(NCORES):
        s = slice(c * DH, (c + 1) * DH)
        p1 = np.concatenate([
            (wq_cross[s] * nq_w[None, :] * scale).T,
            (wk_self[s] * nq_w[None, :]).T], axis=1)
        p2 = np.concatenate([
            (wv_self[s] * nq_w[None, :]).T,
            (wq_self[s] * nq_w[None, :] * scale).T], axis=1)
        p3 = np.concatenate([
            (wk_cross[s] * nkv_w[None, :]).T,
            (wq_self[s] * nkv_w[None, :] * scale).T], axis=1)
        p4 = np.concatenate([
            (wv_cross[s] * nkv_w[None, :]).T,
            (wk_self[s] * nkv_w[None, :]).T], axis=1)
        p5 = (wv_self[s] * nkv_w[None, :]).T
        pall = np.concatenate([_shuf(p1, 8), _shuf(p2, 8), _shuf(p3, 8),
                               _shuf(p4, 8), _shuf(p5, 8)], axis=2)
        in_maps1.append({"qnT": qnT, "kvnT": kvnT, "pw": _bf16(pall)})
    res1 = _run(nc1, in_maps1, "l1")
    # outputs come back [128, 16, 64]; rows t = tb*128 + p
    unblk = lambda a: np.ascontiguousarray(
        a.transpose(1, 0, 2).reshape(T, DH))
    self_out = np.concatenate(
        [unblk(res1.results[c]["self_o"]) for c in range(NCORES)], axis=1)
    cross_out = np.concatenate(
        [unblk(res1.results[c]["cross_o"]) for c in range(NCORES)], axis=1)
    wt_ctx = np.concatenate(
        [unblk(res1.results[c]["wt_o"]) for c in range(NCORES)], axis=1)

    # ---------------- launch 2
    wq_g = mha_in_w[0:INNER]
    wk_g = mha_in_w[INNER:2 * INNER]
    wv_g = mha_in_w[2 * INNER:3 * INNER]
    dmix = mix_w[1] - mix_w[0]
    bdiff = float(mix_b[1] - mix_b[0])
    mvec = (mha_out_w.T @ dmix).reshape(INNER)
    mv2 = np.zeros((65, 8, 2), np.float32)
    for h in range(H):
        mv2[0:64, h, 0] = mvec[h * 64:(h + 1) * 64]
        mv2[64, h, 1] = 1.0
    gw8 = _fp8(np.concatenate([
        _shuf((wk_g * gn_w[None, :]).T * 64.0, 4),
        _shuf((wq_g * gn_w[None, :] * scale).T * 64.0, 4),
        _shuf((wv_g * gn_w[None, :]).T * 64.0, 4)], axis=2))
    woT = _bf16(_shuf(w_out.T, 4))
    wf1s = (ff_fc1 * ff_ln_w[None, :]).T          # [D, FF]
    wf1s = wf1s.reshape(8, 128, 8, 512).transpose(2, 1, 0, 3)  # [g,p,kc,n]
    wf2s = (ff_fc2 * float(ff_gate.reshape(-1)[0])).T          # [FF, D]
    wf2s = wf2s.reshape(8, 4, 128, D).transpose(0, 2, 1, 3)    # [g,p,mo,n]

    nc2 = _get(("l2", bdiff), build_l2, bdiff)
    in_maps2 = []
    wf1sb = _bf16(wf1s)
    wf2sb = _bf16(wf2s)
    mv2b = _bf16(mv2)
    for c in range(NCORES):
        g0 = c * RPC
        bb = g0 // N
        rows3 = np.concatenate([self_out[g0:g0 + RPC],
                                cross_out[g0:g0 + RPC],
                                wt_ctx[g0:g0 + RPC]], axis=0)
        in_maps2.append({
            "rows3d": np.ascontiguousarray(rows3),
            "crossb": cross_out[bb * N:(bb + 1) * N],
            "gw8": gw8, "mv2d": mv2b, "woT": woT,
            "wf1T": wf1sb, "wf2T": wf2sb,
        })
    res2 = _run(nc2, in_maps2, "l2")
    delta = np.concatenate(
        [res2.results[c]["outd"] for c in range(NCORES)], axis=0)
    wt_out = np.concatenate(
        [res2.results[c]["outw"] for c in range(NCORES)], axis=0)

    return np.stack([delta.reshape(B, N, D),
                     wt_out.reshape(B, N, D)]).astype(np.float32)
